# revision 18
# baseline (speedup 1.0000x reference)
"""MultiConditionCrossAttention Trainium2 kernel (8 NeuronCores, data-parallel over B).

Math (per batch b):
    q = x @ w_q.T                                  (B, N, 512)
    kv = conditions @ w_kv.T -> k, v               (B, C=16, H=8, hd=64)
    S = einsum('nhd,chd->hnc', q, k) * SCALE       masked softmax over c
    out = einsum('hnc,chd->nhd', attn, v) @ w_proj.T + b_proj

Key restructuring (exact algebra, see kernel_baseline_222us.py.bak for the
f32 ancestor):
  - Block layouts: K_blk[16h+c, :] = k[c,h,:] placed in head-h's 64-col slice
    (zeros elsewhere); V_blk likewise. Then for all heads at once:
        S_all[n, 16h+c] = q[n] @ K_blk[16h+c]        (block-diag trick)
        out[n]          = attn_all[n] @ V_blk @ w_proj.T + b
  - Weight folding (HOST, float64; q only feeds S, V_blk only feeds the
    projection):
        W_s   = SCALE * K_blk @ w_q          [128, 512]   (per b)
        W_v2p = V_blk @ w_proj.T + b_proj/8  [128, 512]   (per b)
    using sum_ch attn_all[n, ch] = H = 8 to fold the bias exactly.
  - bf16 I/O (the big change vs the f32 baseline): x is cast to bf16 AND
    pre-transposed on the host into chunk-major x^T layout
    xt[b, ci, p, kt, n] = x[b, ci*1024+n, kt*128+p], so the device does NO
    transposes at all and loads half the bytes. y is stored bf16 and
    widened on the host. All weights are bf16; accumulation stays f32 in
    PSUM. Numpy-simulated end-to-end rel err of the full bf16 pipeline:
    4.5e-3 vs the fp32 jax reference (gate is 2e-2); measured on HW:
    4.52e-3. This halves DMA traffic 64MB -> 32MB per core per iteration;
    the f32 baseline was DMA-roofline-bound at ~223us (~300GB/s effective
    of the ~358GB/s per-core HBM limit).

Per 1024-token chunk (16 chunks per core per iteration):
    load xT bf16 [128, 4, 1024]            (1 MB DMA, sync/HWDGE)
    S^T  = W_s @ x^T   (8 bf16 matmuls into [128,1024] f32 PSUM, K=512)
    E    = exp(S^T + mask_bias)            (1 ACT op, PSUM->SBUF bf16)
    Zrep = blk16.T @ E                     (2 matmuls -> per-head sums)
    A    = E * recip_approx(Zrep)          (DVE recip f32 + mul)
    y    = A^T n-slices @ W_v2p            (8 matmuls + 8 PSUM->SBUF copies)
    store y bf16 [128, 8, 512]             (1 MB DMA, scalar/HWDGE)

Measured (8x trn2 NeuronCores, axon; device-resident-input For_i slope,
R=513->2049, see fastbench.py): 120.2us/iteration steady state with the
default knobs (act_copies=5, bufs_s=1, bufs_y=2, bufs_x=4, bufs_sm=4),
vs 222.9us for the f32 baseline (kernel_baseline_222us.py.bak).

HW attribution (For_i slope of skip-variants, default-era config):
full 131.8 / no-store 115.8 / no-softmax 125.9 / compute-only 109.2 /
compute-only minus 6 of 8 y-copies 80.0 us. I.e. after bf16 halves the
DMA, the PSUM->SBUF y-copy + exp/recip/mul pipeline on ACT+DVE is
co-critical with DMA, and most of the win beyond that came from PSUM
pressure (bufs_s 2->1) + copy rebalance (act_copies 4->5).

Negative results (measured, don't retry blindly): 2-deep software skew
(PE stream S(i), zb(i-1), Y(i-2)) 141.5us vs 120.2 plain — same
conclusion as the f32 session's 1-deep skew; sim (TimelineSim) deltas do
NOT transfer to HW (sim favored bufs_s=1+bufs_y=4 which HW ranks worse
than bufs_s=1+bufs_y=2); Y-matmuls with wv2p stationary (y^T layout,
halves LDWEIGHTS) is a wash (129.0 vs 129.2); deeper SBUF pools beyond
(x4, sm4, ysb3) are a wash; fp8 x fails the rel-err gate (2.9e-2
numpy-sim vs 2e-2 gate).
"""

import os
import numpy as np
import ml_dtypes

import concourse.mybir as mybir
import concourse.tile as tile
from concourse import bacc
from concourse.bass_utils import run_bass_kernel_spmd

F32 = mybir.dt.float32
BF16 = mybir.dt.bfloat16
NP_BF16 = ml_dtypes.bfloat16

N_CORES = 8
B, N, D = 16, 8192, 512
C, H, HD = 16, 8, 64
COND_DIM = 256
SCALE = (D // H) ** -0.5
B_PER_CORE = B // N_CORES          # 2
CHUNK = 1024                       # tokens per chunk
CHUNKS_PER_B = N // CHUNK          # 8
GRPS = CHUNK // 128                # 8 n-groups per chunk
NEG = -60.0                        # mask bias (exp(-60+s) ~ 0)

# Y-matmul orientation: False = a_r slices stationary, y n-major
# (y_dev[b,ci,p,g,d], token = ci*CHUNK + g*128 + p);  True = wv2p slices
# stationary (reused across both chunk halves), y feature-major
# (y_dev[b,ci,p,dt,n], y[b, ci*CHUNK+n, dt*128+p]).
Y_WSTAT = os.environ.get("MCCA_YWSTAT", "0") == "1"

_cache = {}


def _build(repeat=1, bufs_x=4, bufs_sm=4, bufs_ysb=3, bufs_s=1, bufs_zb=1,
           bufs_y=2, skip=(), staggered=True, kt_outer=False, act_copies=5,
           skew=False):
    if skew:
        return _build_skew(repeat=repeat, bufs_x=bufs_x, bufs_sm=bufs_sm,
                           bufs_ysb=bufs_ysb, bufs_s=bufs_s, bufs_zb=bufs_zb,
                           bufs_y=bufs_y, skip=skip, staggered=staggered,
                           act_copies=act_copies)
    return _build_plain(repeat=repeat, bufs_x=bufs_x, bufs_sm=bufs_sm,
                        bufs_ysb=bufs_ysb, bufs_s=bufs_s, bufs_zb=bufs_zb,
                        bufs_y=bufs_y, skip=skip, staggered=staggered,
                        kt_outer=kt_outer, act_copies=act_copies)


def _build_plain(repeat=1, bufs_x=3, bufs_sm=3, bufs_ysb=3, bufs_s=2,
                 bufs_zb=1, bufs_y=2, skip=(), staggered=True, kt_outer=False,
                 act_copies=4):
    nc = bacc.Bacc("TRN2", target_bir_lowering=False, debug=False,
                   num_devices=N_CORES)

    xt_d = nc.dram_tensor("xt", [B_PER_CORE, CHUNKS_PER_B, 128, 4, CHUNK],
                          BF16, kind="ExternalInput").ap()
    wsT_d = nc.dram_tensor("wsT", [B_PER_CORE, 128, 4, 128], BF16,
                           kind="ExternalInput").ap()
    wv2p_d = nc.dram_tensor("wv2p", [B_PER_CORE, 128, D], BF16,
                            kind="ExternalInput").ap()
    maskb_d = nc.dram_tensor("mask_bias", [B_PER_CORE, 128, 1], F32,
                             kind="ExternalInput").ap()
    blk16_d = nc.dram_tensor("blk16", [128, 128], BF16,
                             kind="ExternalInput").ap()
    y_shape = ([B_PER_CORE, CHUNKS_PER_B, 128, 4, CHUNK] if Y_WSTAT
               else [B_PER_CORE, CHUNKS_PER_B, 128, GRPS, D])
    y_d = nc.dram_tensor("y", y_shape, BF16, kind="ExternalOutput").ap()

    from contextlib import ExitStack
    with tile.TileContext(nc) as tc:
        with ExitStack() as stack:
            cp = stack.enter_context(tc.tile_pool(name="const", bufs=1))
            wsT = []
            wv2p = []
            for b in range(B_PER_CORE):
                w = cp.tile([128, 4, 128], BF16, tag=f"wsT{b}")
                nc.scalar.dma_start(w[:], wsT_d[b])
                wsT.append(w)
            for b in range(B_PER_CORE):
                w = cp.tile([128, D], BF16, tag=f"wv2p{b}")
                nc.scalar.dma_start(w[:], wv2p_d[b])
                wv2p.append(w)
            blk16 = cp.tile([128, 128], BF16, tag="blk16")
            nc.scalar.dma_start(blk16[:], blk16_d[:])
            maskb = []
            for b in range(B_PER_CORE):
                m = cp.tile([128, 1], F32, tag=f"maskb{b}")
                nc.scalar.dma_start(m[:], maskb_d[b])
                maskb.append(m)

            # ---------------- main loop ----------------
            with (
                tc.tile_pool(name="m_x", bufs=bufs_x) as mp_x,
                tc.tile_pool(name="m_sm", bufs=bufs_sm) as mp_s,
                tc.tile_pool(name="m_ys", bufs=bufs_ysb) as mp_y,
                tc.tile_pool(name="ps_s", bufs=bufs_s, space="PSUM") as ps_s,
                tc.tile_pool(name="ps_zb", bufs=bufs_zb, space="PSUM") as ps_zb,
                tc.tile_pool(name="ps_y", bufs=bufs_y, space="PSUM") as ps_y,
            ):
                from contextlib import nullcontext
                rep_ctx = (tc.For_i(0, repeat, 1, staggered_reset=staggered)
                           if repeat > 1 else nullcontext())
                with rep_ctx:
                    for b in range(B_PER_CORE):
                        for ci in range(CHUNKS_PER_B):
                            xT = mp_x.tile([128, 4, CHUNK], BF16, tag="xT")
                            if "load" not in skip:
                                nc.sync.dma_start(xT[:], xt_d[b, ci])
                            else:
                                nc.vector.memset(xT[:, 0, 0:4], 0.0)

                            # S^T = W_s @ x^T  [128 ch, 1024 n] f32 PSUM
                            s_ps = ps_s.tile([128, CHUNK], F32, tag="s_ps")
                            if kt_outer:
                                # each wsT k-tile stationary for 2 matmuls
                                for kt in range(4):
                                    for hf in range(2):
                                        sl = slice(hf * 512, (hf + 1) * 512)
                                        nc.tensor.matmul(
                                            s_ps[:, sl], wsT[b][:, kt, :],
                                            xT[:, kt, sl],
                                            start=(kt == 0), stop=(kt == 3))
                            else:
                                for hf in range(2):
                                    sl = slice(hf * 512, (hf + 1) * 512)
                                    for kt in range(4):
                                        nc.tensor.matmul(
                                            s_ps[:, sl], wsT[b][:, kt, :],
                                            xT[:, kt, sl],
                                            start=(kt == 0), stop=(kt == 3))

                            # E = exp(S + mask_bias), bf16
                            e_r = mp_s.tile([128, CHUNK], BF16, tag="e_r")
                            nc.scalar.activation(
                                e_r[:], s_ps[:],
                                mybir.ActivationFunctionType.Exp,
                                bias=maskb[b][:], scale=1.0)

                            a_r = mp_s.tile([128, CHUNK], BF16, tag="a_r")
                            if "softmax" not in skip:
                                # Zrep[ch, n] = per-head sum of E, replicated
                                zb_ps = ps_zb.tile([128, CHUNK], F32,
                                                   tag="zb_ps")
                                for hf in range(2):
                                    sl = slice(hf * 512, (hf + 1) * 512)
                                    nc.tensor.matmul(zb_ps[:, sl], blk16[:],
                                                     e_r[:, sl],
                                                     start=True, stop=True)
                                rzb = mp_s.tile([128, CHUNK], F32, tag="rzb")
                                nc.vector.reciprocal_approx_fast(rzb[:],
                                                                 zb_ps[:])
                                # A = E * recip(Zrep)
                                nc.vector.tensor_mul(a_r[:], e_r[:], rzb[:])
                            else:
                                nc.vector.tensor_copy(a_r[:], e_r[:])

                            act_set = {round(i * GRPS / act_copies)
                                       for i in range(act_copies)} \
                                if act_copies > 0 else set()
                            if Y_WSTAT:
                                # y^T[dout, n] = W_v2p^T @ A: wv2p dt-slice is
                                # stationary for both chunk halves (half the
                                # LDWEIGHTS traffic), a_r is the moving operand
                                y_sb = mp_y.tile([128, 4, CHUNK], BF16,
                                                 tag="y_sb")
                                for dt in range(4):
                                    wsl = wv2p[b][:, dt * 128:(dt + 1) * 128]
                                    for hf in range(2):
                                        g = dt * 2 + hf
                                        sl = slice(hf * 512, (hf + 1) * 512)
                                        y_ps = ps_y.tile([128, D], F32,
                                                         tag="y_ps")
                                        nc.tensor.matmul(y_ps[:], wsl,
                                                         a_r[:, sl],
                                                         start=True, stop=True)
                                        if "ycopy" in skip and g >= 2:
                                            continue
                                        if g in act_set:
                                            nc.scalar.copy(
                                                y_sb[:, dt, sl], y_ps[:])
                                        else:
                                            nc.vector.tensor_copy(
                                                y_sb[:, dt, sl], y_ps[:])
                            else:
                                # y[n-grp g] = A[:, g].T @ W_v2p -> [128 n, 512]
                                y_sb = mp_y.tile([128, GRPS, D], BF16,
                                                 tag="y_sb")
                                for g in range(GRPS):
                                    y_ps = ps_y.tile([128, D], F32, tag="y_ps")
                                    nc.tensor.matmul(
                                        y_ps[:], a_r[:, g * 128:(g + 1) * 128],
                                        wv2p[b][:], start=True, stop=True)
                                    # `act_copies` of 8 go to ScalarE (from
                                    # g=0), spread evenly; rest to VectorE
                                    if "ycopy" in skip and g >= 2:
                                        continue
                                    if g in act_set:
                                        nc.scalar.copy(y_sb[:, g, :], y_ps[:])
                                    else:
                                        nc.vector.tensor_copy(y_sb[:, g, :],
                                                              y_ps[:])

                            if "store" not in skip:
                                nc.scalar.dma_start(y_d[b, ci], y_sb[:])

    nc.compile()
    return nc


def _build_skew(repeat=1, bufs_x=3, bufs_sm=4, bufs_ysb=3, bufs_s=2,
                bufs_zb=1, bufs_y=2, skip=(), staggered=True, act_copies=4):
    """Two-deep software-pipelined variant: emission iteration i runs
    S-matmuls(i) / exp(i), softmax zb+recip+mul(i-1), Y-matmuls+copies+
    store(i-2). Every engine-stream instruction then depends only on work
    from previous emission iterations, so the in-order PE/ACT/DVE queues
    never stall on the intra-chunk exp->zb->recip->mul->Y chain."""
    nc = bacc.Bacc("TRN2", target_bir_lowering=False, debug=False,
                   num_devices=N_CORES)

    xt_d = nc.dram_tensor("xt", [B_PER_CORE, CHUNKS_PER_B, 128, 4, CHUNK],
                          BF16, kind="ExternalInput").ap()
    wsT_d = nc.dram_tensor("wsT", [B_PER_CORE, 128, 4, 128], BF16,
                           kind="ExternalInput").ap()
    wv2p_d = nc.dram_tensor("wv2p", [B_PER_CORE, 128, D], BF16,
                            kind="ExternalInput").ap()
    maskb_d = nc.dram_tensor("mask_bias", [B_PER_CORE, 128, 1], F32,
                             kind="ExternalInput").ap()
    blk16_d = nc.dram_tensor("blk16", [128, 128], BF16,
                             kind="ExternalInput").ap()
    assert not Y_WSTAT, "skew builder only implements the n-major y layout"
    y_d = nc.dram_tensor("y", [B_PER_CORE, CHUNKS_PER_B, 128, GRPS, D],
                         BF16, kind="ExternalOutput").ap()

    from contextlib import ExitStack
    with tile.TileContext(nc) as tc:
        with ExitStack() as stack:
            cp = stack.enter_context(tc.tile_pool(name="const", bufs=1))
            wsT = []
            wv2p = []
            for b in range(B_PER_CORE):
                w = cp.tile([128, 4, 128], BF16, tag=f"wsT{b}")
                nc.scalar.dma_start(w[:], wsT_d[b])
                wsT.append(w)
            for b in range(B_PER_CORE):
                w = cp.tile([128, D], BF16, tag=f"wv2p{b}")
                nc.scalar.dma_start(w[:], wv2p_d[b])
                wv2p.append(w)
            blk16 = cp.tile([128, 128], BF16, tag="blk16")
            nc.scalar.dma_start(blk16[:], blk16_d[:])
            maskb = []
            for b in range(B_PER_CORE):
                m = cp.tile([128, 1], F32, tag=f"maskb{b}")
                nc.scalar.dma_start(m[:], maskb_d[b])
                maskb.append(m)

            chunks = [(b, ci) for b in range(B_PER_CORE)
                      for ci in range(CHUNKS_PER_B)]
            n_ch = len(chunks)
            act_set = {round(i * GRPS / act_copies)
                       for i in range(act_copies)} if act_copies > 0 else set()

            with (
                tc.tile_pool(name="m_x", bufs=bufs_x) as mp_x,
                tc.tile_pool(name="m_sm", bufs=bufs_sm) as mp_s,
                tc.tile_pool(name="m_ys", bufs=bufs_ysb) as mp_y,
                tc.tile_pool(name="ps_s", bufs=bufs_s, space="PSUM") as ps_s,
                tc.tile_pool(name="ps_zb", bufs=bufs_zb, space="PSUM") as ps_zb,
                tc.tile_pool(name="ps_y", bufs=bufs_y, space="PSUM") as ps_y,
            ):
                from contextlib import nullcontext
                rep_ctx = (tc.For_i(0, repeat, 1, staggered_reset=staggered)
                           if repeat > 1 else nullcontext())
                with rep_ctx:
                    state = {}
                    for idx in range(n_ch + 2):
                        # ---- stage A: load + S + exp for chunk idx ----
                        if idx < n_ch:
                            b, ci = chunks[idx]
                            xT = mp_x.tile([128, 4, CHUNK], BF16, tag="xT")
                            if "load" not in skip:
                                nc.sync.dma_start(xT[:], xt_d[b, ci])
                            else:
                                nc.vector.memset(xT[:, 0, 0:4], 0.0)
                            s_ps = ps_s.tile([128, CHUNK], F32, tag="s_ps")
                            for hf in range(2):
                                sl = slice(hf * 512, (hf + 1) * 512)
                                for kt in range(4):
                                    nc.tensor.matmul(
                                        s_ps[:, sl], wsT[b][:, kt, :],
                                        xT[:, kt, sl],
                                        start=(kt == 0), stop=(kt == 3))
                            e_r = mp_s.tile([128, CHUNK], BF16, tag="e_r")
                            nc.scalar.activation(
                                e_r[:], s_ps[:],
                                mybir.ActivationFunctionType.Exp,
                                bias=maskb[b][:], scale=1.0)
                            state[idx] = dict(b=b, ci=ci, e_r=e_r)

                        # ---- stage B: zb + recip + mul for chunk idx-1 ----
                        if 0 <= idx - 1 < n_ch:
                            st = state[idx - 1]
                            e1 = st["e_r"]
                            zb_ps = ps_zb.tile([128, CHUNK], F32, tag="zb_ps")
                            for hf in range(2):
                                sl = slice(hf * 512, (hf + 1) * 512)
                                nc.tensor.matmul(zb_ps[:, sl], blk16[:],
                                                 e1[:, sl],
                                                 start=True, stop=True)
                            a_r = mp_s.tile([128, CHUNK], BF16, tag="a_r")
                            if "softmax" not in skip:
                                rzb = mp_s.tile([128, CHUNK], F32, tag="rzb")
                                nc.vector.reciprocal_approx_fast(rzb[:],
                                                                 zb_ps[:])
                                nc.vector.tensor_mul(a_r[:], e1[:], rzb[:])
                            else:
                                nc.vector.tensor_copy(a_r[:], e1[:])
                            st["a_r"] = a_r

                        # ---- stage C: Y + copies + store for chunk idx-2 ----
                        if idx - 2 >= 0:
                            st = state.pop(idx - 2)
                            b2 = st["b"]
                            a2 = st["a_r"]
                            y_sb = mp_y.tile([128, GRPS, D], BF16, tag="y_sb")
                            for g in range(GRPS):
                                y_ps = ps_y.tile([128, D], F32, tag="y_ps")
                                nc.tensor.matmul(
                                    y_ps[:], a2[:, g * 128:(g + 1) * 128],
                                    wv2p[b2][:], start=True, stop=True)
                                if "ycopy" in skip and g >= 2:
                                    continue
                                if g in act_set:
                                    nc.scalar.copy(y_sb[:, g, :], y_ps[:])
                                else:
                                    nc.vector.tensor_copy(y_sb[:, g, :],
                                                          y_ps[:])
                            if "store" not in skip:
                                nc.scalar.dma_start(y_d[st["b"], st["ci"]],
                                                    y_sb[:])

    nc.compile()
    return nc


def _prep_inputs(x, conditions, condition_mask, w_q, w_kv, w_proj, b_proj):
    """Host-side marshalling: shard over B, fold per-batch weights (f64),
    cast everything to bf16, pre-transpose x into chunk-major x^T layout."""
    x = np.asarray(x, dtype=np.float32)
    conditions = np.asarray(conditions, dtype=np.float64)
    condition_mask = np.asarray(condition_mask)
    w_q = np.asarray(w_q, dtype=np.float64)
    w_kv = np.asarray(w_kv, dtype=np.float64)
    w_proj = np.asarray(w_proj, dtype=np.float64)
    b_proj = np.asarray(b_proj, dtype=np.float64)

    # kv projection for all batches: [B, C, 2, H, hd]
    kv = (conditions @ w_kv.T).reshape(B, C, 2, H, HD)
    k = kv[:, :, 0]    # [B, C, H, hd]
    v = kv[:, :, 1]

    blk16 = np.zeros((128, 128), dtype=NP_BF16)
    for h in range(H):
        blk16[h * C:(h + 1) * C, h * C:(h + 1) * C] = 1.0

    wsT_all = np.zeros((B, 128, 4, 128), dtype=NP_BF16)
    wv2p_all = np.zeros((B, 128, D), dtype=NP_BF16)
    for b in range(B):
        K_blk = np.zeros((128, D))
        V_blk = np.zeros((128, D))
        for h in range(H):
            K_blk[h * C:(h + 1) * C, h * HD:(h + 1) * HD] = k[b, :, h, :]
            V_blk[h * C:(h + 1) * C, h * HD:(h + 1) * HD] = v[b, :, h, :]
        W_s = SCALE * (K_blk @ w_q)             # [ch, ki]
        # lhsT tile layout [ki_in_tile, kt, ch]: wsT[p, t, c] = W_s[c, t*128+p]
        wsT_all[b] = W_s.T.reshape(4, 128, 128).transpose(1, 0, 2).astype(
            NP_BF16)
        wv2p_all[b] = (V_blk @ w_proj.T + b_proj[None, :] / H).astype(NP_BF16)

    # x^T chunks: xt[b, ci, p, kt, n] = x[b, ci*CHUNK + n, kt*128 + p], bf16
    xt_all = np.ascontiguousarray(
        x.reshape(B, CHUNKS_PER_B, CHUNK, 4, 128).transpose(0, 1, 4, 3, 2)
    ).astype(NP_BF16)

    in_maps = []
    for core in range(N_CORES):
        b0 = core * B_PER_CORE
        mb = np.zeros((B_PER_CORE, 128, 1), dtype=np.float32)
        for b in range(B_PER_CORE):
            m = condition_mask[b0 + b].astype(bool)          # [16]
            col = np.where(np.tile(m, H), 0.0, NEG).astype(np.float32)
            mb[b, :, 0] = col
        in_maps.append(dict(
            xt=np.ascontiguousarray(xt_all[b0:b0 + B_PER_CORE]),
            wsT=np.ascontiguousarray(wsT_all[b0:b0 + B_PER_CORE]),
            wv2p=np.ascontiguousarray(wv2p_all[b0:b0 + B_PER_CORE]),
            mask_bias=mb,
            blk16=blk16,
        ))
    return in_maps


def _gather(results):
    """Assemble per-core device y layouts back into [B, N, D] f32."""
    y = np.concatenate([r["y"] for r in results], axis=0)
    if Y_WSTAT:
        # y_dev[b, ci, p, dt, n] = y[b, ci*CHUNK + n, dt*128 + p]
        y = y.astype(np.float32).transpose(0, 1, 4, 3, 2).reshape(B, N, D)
    else:
        # y_dev[b, ci, p, g, d] = y[b, ci*CHUNK + g*128 + p, d]
        y = y.astype(np.float32).transpose(0, 1, 3, 2, 4).reshape(B, N, D)
    return np.ascontiguousarray(y)


def kernel(x, conditions, condition_mask, w_q, w_kv, w_proj, b_proj):
    repeat = int(os.environ.get("MCCA_REPEAT", "1"))
    key = ("nc", repeat)
    if key not in _cache:
        _cache[key] = _build(repeat=repeat)
    nc = _cache[key]
    in_maps = _prep_inputs(x, conditions, condition_mask, w_q, w_kv,
                           w_proj, b_proj)
    res = run_bass_kernel_spmd(nc, in_maps, core_ids=list(range(N_CORES)))
    return _gather(res.results)


# revision 21
# speedup vs baseline: 1.2110x; 1.2110x over previous
"""MultiConditionCrossAttention Trainium2 kernel (8 NeuronCores, data-parallel over B).

Math (per batch b):
    q = x @ w_q.T                                  (B, N, 512)
    kv = conditions @ w_kv.T -> k, v               (B, C=16, H=8, hd=64)
    S = einsum('nhd,chd->hnc', q, k) * SCALE       masked softmax over c
    out = einsum('hnc,chd->nhd', attn, v) @ w_proj.T + b_proj

Key restructuring (exact algebra, see kernel_baseline_222us.py.bak for the
f32 ancestor):
  - Block layouts: K_blk[16h+c, :] = k[c,h,:] placed in head-h's 64-col slice
    (zeros elsewhere); V_blk likewise. Then for all heads at once:
        S_all[n, 16h+c] = q[n] @ K_blk[16h+c]        (block-diag trick)
        out[n]          = attn_all[n] @ V_blk @ w_proj.T + b
  - Weight folding (HOST, float64; q only feeds S, V_blk only feeds the
    projection):
        W_s   = SCALE * K_blk @ w_q          [128, 512]   (per b)
        W_v2p = V_blk @ w_proj.T + b_proj/8  [128, 512]   (per b)
    using sum_ch attn_all[n, ch] = H = 8 to fold the bias exactly.
  - bf16 I/O (the big change vs the f32 baseline): x is cast to bf16 AND
    pre-transposed on the host into chunk-major x^T layout
    xt[b, ci, p, kt, n] = x[b, ci*1024+n, kt*128+p], so the device does NO
    transposes at all and loads half the bytes. y is stored bf16 and
    widened on the host. All weights are bf16; accumulation stays f32 in
    PSUM. Numpy-simulated end-to-end rel err of the full bf16 pipeline:
    4.5e-3 vs the fp32 jax reference (gate is 2e-2); measured on HW:
    4.52e-3. This halves DMA traffic 64MB -> 32MB per core per iteration;
    the f32 baseline was DMA-roofline-bound at ~223us (~300GB/s effective
    of the ~358GB/s per-core HBM limit).

Per 1024-token chunk (16 chunks per core per iteration):
    load xT bf16 [128, 4, 1024]            (1 MB DMA, sync/HWDGE)
    S^T  = W_s @ x^T   (8 bf16 matmuls into [128,1024] f32 PSUM, K=512)
    E    = exp(S^T + mask_bias)            (1 ACT op, PSUM->SBUF bf16)
    Zrep = blk16.T @ E                     (2 matmuls -> per-head sums)
    A    = E * recip_approx(Zrep)          (DVE recip f32 + mul)
    y    = A^T n-slices @ W_v2p            (8 matmuls + 8 PSUM->SBUF copies)
    store y bf16 [128, 8, 512]             (1 MB DMA, scalar/HWDGE)

Measured (8x trn2 NeuronCores, axon; device-resident-input For_i slope,
R=513->2049, see fastbench.py): 120.2us/iteration steady state with the
default knobs (act_copies=5, bufs_s=1, bufs_y=2, bufs_x=4, bufs_sm=4),
vs 222.9us for the f32 baseline (kernel_baseline_222us.py.bak).

HW attribution (For_i slope of skip-variants, default-era config):
full 131.8 / no-store 115.8 / no-softmax 125.9 / compute-only 109.2 /
compute-only minus 6 of 8 y-copies 80.0 us. I.e. after bf16 halves the
DMA, the PSUM->SBUF y-copy + exp/recip/mul pipeline on ACT+DVE is
co-critical with DMA, and most of the win beyond that came from PSUM
pressure (bufs_s 2->1) + copy rebalance (act_copies 4->5).

Negative results (measured, don't retry blindly): 2-deep software skew
(PE stream S(i), zb(i-1), Y(i-2)) 141.5us vs 120.2 plain — same
conclusion as the f32 session's 1-deep skew; sim (TimelineSim) deltas do
NOT transfer to HW (sim favored bufs_s=1+bufs_y=4 which HW ranks worse
than bufs_s=1+bufs_y=2); Y-matmuls with wv2p stationary (y^T layout,
halves LDWEIGHTS) is a wash (129.0 vs 129.2); deeper SBUF pools beyond
(x4, sm4, ysb3) are a wash; fp8 x fails the rel-err gate (2.9e-2
numpy-sim vs 2e-2 gate).
"""

import os
import numpy as np
import ml_dtypes

import concourse.mybir as mybir
import concourse.tile as tile
from concourse import bacc
from concourse.bass_utils import run_bass_kernel_spmd

F32 = mybir.dt.float32
BF16 = mybir.dt.bfloat16
NP_BF16 = ml_dtypes.bfloat16

N_CORES = 8
B, N, D = 16, 8192, 512
C, H, HD = 16, 8, 64
COND_DIM = 256
SCALE = (D // H) ** -0.5
B_PER_CORE = B // N_CORES          # 2
CHUNK = 1024                       # tokens per chunk
CHUNKS_PER_B = N // CHUNK          # 8
GRPS = CHUNK // 128                # 8 n-groups per chunk
NEG = -60.0                        # mask bias (exp(-60+s) ~ 0)

# Y-matmul orientation: False = a_r slices stationary, y n-major
# (y_dev[b,ci,p,g,d], token = ci*CHUNK + g*128 + p);  True = wv2p slices
# stationary (reused across both chunk halves), y feature-major
# (y_dev[b,ci,p,dt,n], y[b, ci*CHUNK+n, dt*128+p]).
Y_WSTAT = os.environ.get("MCCA_YWSTAT", "0") == "1"

_cache = {}


def _build(repeat=1, bufs_x=4, bufs_sm=4, bufs_ysb=3, bufs_s=1, bufs_zb=1,
           bufs_y=2, skip=(), staggered=True, kt_outer=False, act_copies=5,
           skew=False, exp_split=False):
    if skew:
        return _build_skew(repeat=repeat, bufs_x=bufs_x, bufs_sm=bufs_sm,
                           bufs_ysb=bufs_ysb, bufs_s=bufs_s, bufs_zb=bufs_zb,
                           bufs_y=bufs_y, skip=skip, staggered=staggered,
                           act_copies=act_copies)
    return _build_plain(repeat=repeat, bufs_x=bufs_x, bufs_sm=bufs_sm,
                        bufs_ysb=bufs_ysb, bufs_s=bufs_s, bufs_zb=bufs_zb,
                        bufs_y=bufs_y, skip=skip, staggered=staggered,
                        kt_outer=kt_outer, act_copies=act_copies,
                        exp_split=exp_split)


def _build_plain(repeat=1, bufs_x=3, bufs_sm=3, bufs_ysb=3, bufs_s=2,
                 bufs_zb=1, bufs_y=2, skip=(), staggered=True, kt_outer=False,
                 act_copies=4, exp_split=False):
    nc = bacc.Bacc("TRN2", target_bir_lowering=False, debug=False,
                   num_devices=N_CORES)

    xt_d = nc.dram_tensor("xt", [B_PER_CORE, CHUNKS_PER_B, 128, 4, CHUNK],
                          BF16, kind="ExternalInput").ap()
    wsT_d = nc.dram_tensor("wsT", [B_PER_CORE, 128, 4, 128], BF16,
                           kind="ExternalInput").ap()
    wv2p_d = nc.dram_tensor("wv2p", [B_PER_CORE, 128, D], BF16,
                            kind="ExternalInput").ap()
    maskb_d = nc.dram_tensor("mask_bias", [B_PER_CORE, 128, 1], F32,
                             kind="ExternalInput").ap()
    blk16_d = nc.dram_tensor("blk16", [128, 128], BF16,
                             kind="ExternalInput").ap()
    y_shape = ([B_PER_CORE, CHUNKS_PER_B, 128, 4, CHUNK] if Y_WSTAT
               else [B_PER_CORE, CHUNKS_PER_B, 128, GRPS, D])
    y_d = nc.dram_tensor("y", y_shape, BF16, kind="ExternalOutput").ap()

    from contextlib import ExitStack
    with tile.TileContext(nc) as tc:
        with ExitStack() as stack:
            cp = stack.enter_context(tc.tile_pool(name="const", bufs=1))
            wsT = []
            wv2p = []
            for b in range(B_PER_CORE):
                w = cp.tile([128, 4, 128], BF16, tag=f"wsT{b}")
                nc.scalar.dma_start(w[:], wsT_d[b])
                wsT.append(w)
            for b in range(B_PER_CORE):
                w = cp.tile([128, D], BF16, tag=f"wv2p{b}")
                nc.scalar.dma_start(w[:], wv2p_d[b])
                wv2p.append(w)
            blk16 = cp.tile([128, 128], BF16, tag="blk16")
            nc.scalar.dma_start(blk16[:], blk16_d[:])
            maskb = []
            for b in range(B_PER_CORE):
                m = cp.tile([128, 1], F32, tag=f"maskb{b}")
                nc.scalar.dma_start(m[:], maskb_d[b])
                maskb.append(m)

            # ---------------- main loop ----------------
            with (
                tc.tile_pool(name="m_x", bufs=bufs_x) as mp_x,
                tc.tile_pool(name="m_sm", bufs=bufs_sm) as mp_s,
                tc.tile_pool(name="m_ys", bufs=bufs_ysb) as mp_y,
                tc.tile_pool(name="ps_s", bufs=bufs_s, space="PSUM") as ps_s,
                tc.tile_pool(name="ps_zb", bufs=bufs_zb, space="PSUM") as ps_zb,
                tc.tile_pool(name="ps_y", bufs=bufs_y, space="PSUM") as ps_y,
            ):
                from contextlib import nullcontext
                rep_ctx = (tc.For_i(0, repeat, 1, staggered_reset=staggered)
                           if repeat > 1 else nullcontext())
                with rep_ctx:
                    for b in range(B_PER_CORE):
                        for ci in range(CHUNKS_PER_B):
                            xT = mp_x.tile([128, 4, CHUNK], BF16, tag="xT")
                            if "load" not in skip:
                                nc.sync.dma_start(xT[:], xt_d[b, ci])
                            else:
                                nc.vector.memset(xT[:, 0, 0:4], 0.0)

                            # S^T = W_s @ x^T  [128 ch, 1024 n] f32 PSUM
                            s_ps = ps_s.tile([128, CHUNK], F32, tag="s_ps")
                            if kt_outer:
                                # each wsT k-tile stationary for 2 matmuls
                                for kt in range(4):
                                    for hf in range(2):
                                        sl = slice(hf * 512, (hf + 1) * 512)
                                        nc.tensor.matmul(
                                            s_ps[:, sl], wsT[b][:, kt, :],
                                            xT[:, kt, sl],
                                            start=(kt == 0), stop=(kt == 3))
                            else:
                                for hf in range(2):
                                    sl = slice(hf * 512, (hf + 1) * 512)
                                    for kt in range(4):
                                        nc.tensor.matmul(
                                            s_ps[:, sl], wsT[b][:, kt, :],
                                            xT[:, kt, sl],
                                            start=(kt == 0), stop=(kt == 3))

                            # E = exp(S + mask_bias), bf16
                            e_r = mp_s.tile([128, CHUNK], BF16, tag="e_r")
                            if exp_split:
                                # per-half exp: half 0 can run as soon as its
                                # 4 S-matmuls finish, so the zb chain starts
                                # ~4 matmuls earlier
                                for hf in range(2):
                                    sl = slice(hf * 512, (hf + 1) * 512)
                                    nc.scalar.activation(
                                        e_r[:, sl], s_ps[:, sl],
                                        mybir.ActivationFunctionType.Exp,
                                        bias=maskb[b][:], scale=1.0)
                            else:
                                nc.scalar.activation(
                                    e_r[:], s_ps[:],
                                    mybir.ActivationFunctionType.Exp,
                                    bias=maskb[b][:], scale=1.0)

                            a_r = mp_s.tile([128, CHUNK], BF16, tag="a_r")
                            if "softmax" not in skip:
                                # Zrep[ch, n] = per-head sum of E, replicated
                                zb_ps = ps_zb.tile([128, CHUNK], F32,
                                                   tag="zb_ps")
                                for hf in range(2):
                                    sl = slice(hf * 512, (hf + 1) * 512)
                                    nc.tensor.matmul(zb_ps[:, sl], blk16[:],
                                                     e_r[:, sl],
                                                     start=True, stop=True)
                                rzb = mp_s.tile([128, CHUNK], F32, tag="rzb")
                                nc.vector.reciprocal_approx_fast(rzb[:],
                                                                 zb_ps[:])
                                # A = E * recip(Zrep)
                                nc.vector.tensor_mul(a_r[:], e_r[:], rzb[:])
                            else:
                                nc.vector.tensor_copy(a_r[:], e_r[:])

                            act_set = {round(i * GRPS / act_copies)
                                       for i in range(act_copies)} \
                                if act_copies > 0 else set()
                            if Y_WSTAT:
                                # y^T[dout, n] = W_v2p^T @ A: wv2p dt-slice is
                                # stationary for both chunk halves (half the
                                # LDWEIGHTS traffic), a_r is the moving operand
                                y_sb = mp_y.tile([128, 4, CHUNK], BF16,
                                                 tag="y_sb")
                                for dt in range(4):
                                    wsl = wv2p[b][:, dt * 128:(dt + 1) * 128]
                                    for hf in range(2):
                                        g = dt * 2 + hf
                                        sl = slice(hf * 512, (hf + 1) * 512)
                                        y_ps = ps_y.tile([128, D], F32,
                                                         tag="y_ps")
                                        nc.tensor.matmul(y_ps[:], wsl,
                                                         a_r[:, sl],
                                                         start=True, stop=True)
                                        if "ycopy" in skip and g >= 2:
                                            continue
                                        if g in act_set:
                                            nc.scalar.copy(
                                                y_sb[:, dt, sl], y_ps[:])
                                        else:
                                            nc.vector.tensor_copy(
                                                y_sb[:, dt, sl], y_ps[:])
                            else:
                                # y[n-grp g] = A[:, g].T @ W_v2p -> [128 n, 512]
                                y_sb = mp_y.tile([128, GRPS, D], BF16,
                                                 tag="y_sb")
                                for g in range(GRPS):
                                    y_ps = ps_y.tile([128, D], F32, tag="y_ps")
                                    nc.tensor.matmul(
                                        y_ps[:], a_r[:, g * 128:(g + 1) * 128],
                                        wv2p[b][:], start=True, stop=True)
                                    # `act_copies` of 8 go to ScalarE (from
                                    # g=0), spread evenly; rest to VectorE
                                    if "ycopy" in skip and g >= 2:
                                        continue
                                    if g in act_set:
                                        nc.scalar.copy(y_sb[:, g, :], y_ps[:])
                                    else:
                                        nc.vector.tensor_copy(y_sb[:, g, :],
                                                              y_ps[:])

                            if "store" not in skip:
                                nc.scalar.dma_start(y_d[b, ci], y_sb[:])

    nc.compile()
    return nc


def _build_skew(repeat=1, bufs_x=3, bufs_sm=4, bufs_ysb=3, bufs_s=2,
                bufs_zb=1, bufs_y=2, skip=(), staggered=True, act_copies=4):
    """Two-deep software-pipelined variant: emission iteration i runs
    S-matmuls(i) / exp(i), softmax zb+recip+mul(i-1), Y-matmuls+copies+
    store(i-2). Every engine-stream instruction then depends only on work
    from previous emission iterations, so the in-order PE/ACT/DVE queues
    never stall on the intra-chunk exp->zb->recip->mul->Y chain."""
    nc = bacc.Bacc("TRN2", target_bir_lowering=False, debug=False,
                   num_devices=N_CORES)

    xt_d = nc.dram_tensor("xt", [B_PER_CORE, CHUNKS_PER_B, 128, 4, CHUNK],
                          BF16, kind="ExternalInput").ap()
    wsT_d = nc.dram_tensor("wsT", [B_PER_CORE, 128, 4, 128], BF16,
                           kind="ExternalInput").ap()
    wv2p_d = nc.dram_tensor("wv2p", [B_PER_CORE, 128, D], BF16,
                            kind="ExternalInput").ap()
    maskb_d = nc.dram_tensor("mask_bias", [B_PER_CORE, 128, 1], F32,
                             kind="ExternalInput").ap()
    blk16_d = nc.dram_tensor("blk16", [128, 128], BF16,
                             kind="ExternalInput").ap()
    assert not Y_WSTAT, "skew builder only implements the n-major y layout"
    y_d = nc.dram_tensor("y", [B_PER_CORE, CHUNKS_PER_B, 128, GRPS, D],
                         BF16, kind="ExternalOutput").ap()

    from contextlib import ExitStack
    with tile.TileContext(nc) as tc:
        with ExitStack() as stack:
            cp = stack.enter_context(tc.tile_pool(name="const", bufs=1))
            wsT = []
            wv2p = []
            for b in range(B_PER_CORE):
                w = cp.tile([128, 4, 128], BF16, tag=f"wsT{b}")
                nc.scalar.dma_start(w[:], wsT_d[b])
                wsT.append(w)
            for b in range(B_PER_CORE):
                w = cp.tile([128, D], BF16, tag=f"wv2p{b}")
                nc.scalar.dma_start(w[:], wv2p_d[b])
                wv2p.append(w)
            blk16 = cp.tile([128, 128], BF16, tag="blk16")
            nc.scalar.dma_start(blk16[:], blk16_d[:])
            maskb = []
            for b in range(B_PER_CORE):
                m = cp.tile([128, 1], F32, tag=f"maskb{b}")
                nc.scalar.dma_start(m[:], maskb_d[b])
                maskb.append(m)

            chunks = [(b, ci) for b in range(B_PER_CORE)
                      for ci in range(CHUNKS_PER_B)]
            n_ch = len(chunks)
            act_set = {round(i * GRPS / act_copies)
                       for i in range(act_copies)} if act_copies > 0 else set()

            with (
                tc.tile_pool(name="m_x", bufs=bufs_x) as mp_x,
                tc.tile_pool(name="m_sm", bufs=bufs_sm) as mp_s,
                tc.tile_pool(name="m_ys", bufs=bufs_ysb) as mp_y,
                tc.tile_pool(name="ps_s", bufs=bufs_s, space="PSUM") as ps_s,
                tc.tile_pool(name="ps_zb", bufs=bufs_zb, space="PSUM") as ps_zb,
                tc.tile_pool(name="ps_y", bufs=bufs_y, space="PSUM") as ps_y,
            ):
                from contextlib import nullcontext
                rep_ctx = (tc.For_i(0, repeat, 1, staggered_reset=staggered)
                           if repeat > 1 else nullcontext())
                with rep_ctx:
                    state = {}
                    for idx in range(n_ch + 2):
                        # ---- stage A: load + S + exp for chunk idx ----
                        if idx < n_ch:
                            b, ci = chunks[idx]
                            xT = mp_x.tile([128, 4, CHUNK], BF16, tag="xT")
                            if "load" not in skip:
                                nc.sync.dma_start(xT[:], xt_d[b, ci])
                            else:
                                nc.vector.memset(xT[:, 0, 0:4], 0.0)
                            s_ps = ps_s.tile([128, CHUNK], F32, tag="s_ps")
                            for hf in range(2):
                                sl = slice(hf * 512, (hf + 1) * 512)
                                for kt in range(4):
                                    nc.tensor.matmul(
                                        s_ps[:, sl], wsT[b][:, kt, :],
                                        xT[:, kt, sl],
                                        start=(kt == 0), stop=(kt == 3))
                            e_r = mp_s.tile([128, CHUNK], BF16, tag="e_r")
                            nc.scalar.activation(
                                e_r[:], s_ps[:],
                                mybir.ActivationFunctionType.Exp,
                                bias=maskb[b][:], scale=1.0)
                            state[idx] = dict(b=b, ci=ci, e_r=e_r)

                        # ---- stage B: zb + recip + mul for chunk idx-1 ----
                        if 0 <= idx - 1 < n_ch:
                            st = state[idx - 1]
                            e1 = st["e_r"]
                            zb_ps = ps_zb.tile([128, CHUNK], F32, tag="zb_ps")
                            for hf in range(2):
                                sl = slice(hf * 512, (hf + 1) * 512)
                                nc.tensor.matmul(zb_ps[:, sl], blk16[:],
                                                 e1[:, sl],
                                                 start=True, stop=True)
                            a_r = mp_s.tile([128, CHUNK], BF16, tag="a_r")
                            if "softmax" not in skip:
                                rzb = mp_s.tile([128, CHUNK], F32, tag="rzb")
                                nc.vector.reciprocal_approx_fast(rzb[:],
                                                                 zb_ps[:])
                                nc.vector.tensor_mul(a_r[:], e1[:], rzb[:])
                            else:
                                nc.vector.tensor_copy(a_r[:], e1[:])
                            st["a_r"] = a_r

                        # ---- stage C: Y + copies + store for chunk idx-2 ----
                        if idx - 2 >= 0:
                            st = state.pop(idx - 2)
                            b2 = st["b"]
                            a2 = st["a_r"]
                            y_sb = mp_y.tile([128, GRPS, D], BF16, tag="y_sb")
                            for g in range(GRPS):
                                y_ps = ps_y.tile([128, D], F32, tag="y_ps")
                                nc.tensor.matmul(
                                    y_ps[:], a2[:, g * 128:(g + 1) * 128],
                                    wv2p[b2][:], start=True, stop=True)
                                if "ycopy" in skip and g >= 2:
                                    continue
                                if g in act_set:
                                    nc.scalar.copy(y_sb[:, g, :], y_ps[:])
                                else:
                                    nc.vector.tensor_copy(y_sb[:, g, :],
                                                          y_ps[:])
                            if "store" not in skip:
                                nc.scalar.dma_start(y_d[st["b"], st["ci"]],
                                                    y_sb[:])

    nc.compile()
    return nc


def _prep_inputs(x, conditions, condition_mask, w_q, w_kv, w_proj, b_proj):
    """Host-side marshalling: shard over B, fold per-batch weights (f64),
    cast everything to bf16, pre-transpose x into chunk-major x^T layout."""
    x = np.asarray(x, dtype=np.float32)
    conditions = np.asarray(conditions, dtype=np.float64)
    condition_mask = np.asarray(condition_mask)
    w_q = np.asarray(w_q, dtype=np.float64)
    w_kv = np.asarray(w_kv, dtype=np.float64)
    w_proj = np.asarray(w_proj, dtype=np.float64)
    b_proj = np.asarray(b_proj, dtype=np.float64)

    # kv projection for all batches: [B, C, 2, H, hd]
    kv = (conditions @ w_kv.T).reshape(B, C, 2, H, HD)
    k = kv[:, :, 0]    # [B, C, H, hd]
    v = kv[:, :, 1]

    blk16 = np.zeros((128, 128), dtype=NP_BF16)
    for h in range(H):
        blk16[h * C:(h + 1) * C, h * C:(h + 1) * C] = 1.0

    wsT_all = np.zeros((B, 128, 4, 128), dtype=NP_BF16)
    wv2p_all = np.zeros((B, 128, D), dtype=NP_BF16)
    for b in range(B):
        K_blk = np.zeros((128, D))
        V_blk = np.zeros((128, D))
        for h in range(H):
            K_blk[h * C:(h + 1) * C, h * HD:(h + 1) * HD] = k[b, :, h, :]
            V_blk[h * C:(h + 1) * C, h * HD:(h + 1) * HD] = v[b, :, h, :]
        W_s = SCALE * (K_blk @ w_q)             # [ch, ki]
        # lhsT tile layout [ki_in_tile, kt, ch]: wsT[p, t, c] = W_s[c, t*128+p]
        wsT_all[b] = W_s.T.reshape(4, 128, 128).transpose(1, 0, 2).astype(
            NP_BF16)
        wv2p_all[b] = (V_blk @ w_proj.T + b_proj[None, :] / H).astype(NP_BF16)

    # x^T chunks: xt[b, ci, p, kt, n] = x[b, ci*CHUNK + n, kt*128 + p], bf16
    xt_all = np.ascontiguousarray(
        x.reshape(B, CHUNKS_PER_B, CHUNK, 4, 128).transpose(0, 1, 4, 3, 2)
    ).astype(NP_BF16)

    in_maps = []
    for core in range(N_CORES):
        b0 = core * B_PER_CORE
        mb = np.zeros((B_PER_CORE, 128, 1), dtype=np.float32)
        for b in range(B_PER_CORE):
            m = condition_mask[b0 + b].astype(bool)          # [16]
            col = np.where(np.tile(m, H), 0.0, NEG).astype(np.float32)
            mb[b, :, 0] = col
        in_maps.append(dict(
            xt=np.ascontiguousarray(xt_all[b0:b0 + B_PER_CORE]),
            wsT=np.ascontiguousarray(wsT_all[b0:b0 + B_PER_CORE]),
            wv2p=np.ascontiguousarray(wv2p_all[b0:b0 + B_PER_CORE]),
            mask_bias=mb,
            blk16=blk16,
        ))
    return in_maps


def _gather(results):
    """Assemble per-core device y layouts back into [B, N, D] f32."""
    y = np.concatenate([r["y"] for r in results], axis=0)
    if Y_WSTAT:
        # y_dev[b, ci, p, dt, n] = y[b, ci*CHUNK + n, dt*128 + p]
        y = y.astype(np.float32).transpose(0, 1, 4, 3, 2).reshape(B, N, D)
    else:
        # y_dev[b, ci, p, g, d] = y[b, ci*CHUNK + g*128 + p, d]
        y = y.astype(np.float32).transpose(0, 1, 3, 2, 4).reshape(B, N, D)
    return np.ascontiguousarray(y)


def kernel(x, conditions, condition_mask, w_q, w_kv, w_proj, b_proj):
    repeat = int(os.environ.get("MCCA_REPEAT", "1"))
    key = ("nc", repeat)
    if key not in _cache:
        _cache[key] = _build(repeat=repeat)
    nc = _cache[key]
    in_maps = _prep_inputs(x, conditions, condition_mask, w_q, w_kv,
                           w_proj, b_proj)
    res = run_bass_kernel_spmd(nc, in_maps, core_ids=list(range(N_CORES)))
    return _gather(res.results)


# revision 25
# speedup vs baseline: 1.2286x; 1.0145x over previous
"""MultiConditionCrossAttention Trainium2 kernel (8 NeuronCores, data-parallel over B).

Math (per batch b):
    q = x @ w_q.T                                  (B, N, 512)
    kv = conditions @ w_kv.T -> k, v               (B, C=16, H=8, hd=64)
    S = einsum('nhd,chd->hnc', q, k) * SCALE       masked softmax over c
    out = einsum('hnc,chd->nhd', attn, v) @ w_proj.T + b_proj

Key restructuring (exact algebra, see kernel_baseline_222us.py.bak for the
f32 ancestor):
  - Block layouts: K_blk[16h+c, :] = k[c,h,:] placed in head-h's 64-col slice
    (zeros elsewhere); V_blk likewise. Then for all heads at once:
        S_all[n, 16h+c] = q[n] @ K_blk[16h+c]        (block-diag trick)
        out[n]          = attn_all[n] @ V_blk @ w_proj.T + b
  - Weight folding (HOST, float64; q only feeds S, V_blk only feeds the
    projection):
        W_s   = SCALE * K_blk @ w_q          [128, 512]   (per b)
        W_v2p = V_blk @ w_proj.T + b_proj/8  [128, 512]   (per b)
    using sum_ch attn_all[n, ch] = H = 8 to fold the bias exactly.
  - bf16 I/O (the big change vs the f32 baseline): x is cast to bf16 AND
    pre-transposed on the host into chunk-major x^T layout
    xt[b, ci, p, kt, n] = x[b, ci*1024+n, kt*128+p], so the device does NO
    transposes at all and loads half the bytes. y is stored bf16 and
    widened on the host. All weights are bf16; accumulation stays f32 in
    PSUM. Numpy-simulated end-to-end rel err of the full bf16 pipeline:
    4.5e-3 vs the fp32 jax reference (gate is 2e-2); measured on HW:
    4.52e-3. This halves DMA traffic 64MB -> 32MB per core per iteration;
    the f32 baseline was DMA-roofline-bound at ~223us (~300GB/s effective
    of the ~358GB/s per-core HBM limit).

Per 1024-token chunk (16 chunks per core per iteration):
    load xT bf16 [128, 4, 1024]            (1 MB DMA, sync/HWDGE)
    S^T  = W_s @ x^T   (8 bf16 matmuls into [128,1024] f32 PSUM, K=512)
    E    = exp(S^T + mask_bias)            (1 ACT op, PSUM->SBUF bf16)
    Zrep = blk16.T @ E                     (2 matmuls -> per-head sums)
    A    = E * recip_approx(Zrep)          (DVE recip f32 + mul)
    y    = A^T n-slices @ W_v2p            (8 matmuls + 8 PSUM->SBUF copies)
    store y bf16 [128, 8, 512]             (1 MB DMA, scalar/HWDGE)

Measured (8x trn2 NeuronCores, axon; device-resident-input For_i slope,
R=513->2049, see fastbench.py): 120.2us/iteration steady state with the
default knobs (act_copies=5, bufs_s=1, bufs_y=2, bufs_x=4, bufs_sm=4),
vs 222.9us for the f32 baseline (kernel_baseline_222us.py.bak).

HW attribution (For_i slope of skip-variants, default-era config):
full 131.8 / no-store 115.8 / no-softmax 125.9 / compute-only 109.2 /
compute-only minus 6 of 8 y-copies 80.0 us. I.e. after bf16 halves the
DMA, the PSUM->SBUF y-copy + exp/recip/mul pipeline on ACT+DVE is
co-critical with DMA, and most of the win beyond that came from PSUM
pressure (bufs_s 2->1) + copy rebalance (act_copies 4->5).

Negative results (measured, don't retry blindly): 2-deep software skew
(PE stream S(i), zb(i-1), Y(i-2)) 141.5us vs 120.2 plain — same
conclusion as the f32 session's 1-deep skew; sim (TimelineSim) deltas do
NOT transfer to HW (sim favored bufs_s=1+bufs_y=4 which HW ranks worse
than bufs_s=1+bufs_y=2); Y-matmuls with wv2p stationary (y^T layout,
halves LDWEIGHTS) is a wash (129.0 vs 129.2); deeper SBUF pools beyond
(x4, sm4, ysb3) are a wash; fp8 x fails the rel-err gate (2.9e-2
numpy-sim vs 2e-2 gate).
"""

import os
import numpy as np
import ml_dtypes

import concourse.mybir as mybir
import concourse.tile as tile
from concourse import bacc
from concourse.bass_utils import run_bass_kernel_spmd

F32 = mybir.dt.float32
BF16 = mybir.dt.bfloat16
NP_BF16 = ml_dtypes.bfloat16

N_CORES = 8
B, N, D = 16, 8192, 512
C, H, HD = 16, 8, 64
COND_DIM = 256
SCALE = (D // H) ** -0.5
B_PER_CORE = B // N_CORES          # 2
CHUNK = 1024                       # tokens per chunk
CHUNKS_PER_B = N // CHUNK          # 8
GRPS = CHUNK // 128                # 8 n-groups per chunk
NEG = -60.0                        # mask bias (exp(-60+s) ~ 0)

# Y-matmul orientation: False = a_r slices stationary, y n-major
# (y_dev[b,ci,p,g,d], token = ci*CHUNK + g*128 + p);  True = wv2p slices
# stationary (reused across both chunk halves), y feature-major
# (y_dev[b,ci,p,dt,n], y[b, ci*CHUNK+n, dt*128+p]).
Y_WSTAT = os.environ.get("MCCA_YWSTAT", "0") == "1"

_cache = {}


def _build(repeat=1, bufs_x=4, bufs_sm=4, bufs_ysb=3, bufs_s=1, bufs_zb=1,
           bufs_y=2, skip=(), staggered=True, kt_outer=False, act_copies=5,
           skew=False, exp_split=False, load_split=False):
    if skew:
        return _build_skew(repeat=repeat, bufs_x=bufs_x, bufs_sm=bufs_sm,
                           bufs_ysb=bufs_ysb, bufs_s=bufs_s, bufs_zb=bufs_zb,
                           bufs_y=bufs_y, skip=skip, staggered=staggered,
                           act_copies=act_copies)
    return _build_plain(repeat=repeat, bufs_x=bufs_x, bufs_sm=bufs_sm,
                        bufs_ysb=bufs_ysb, bufs_s=bufs_s, bufs_zb=bufs_zb,
                        bufs_y=bufs_y, skip=skip, staggered=staggered,
                        kt_outer=kt_outer, act_copies=act_copies,
                        exp_split=exp_split, load_split=load_split)


def _build_plain(repeat=1, bufs_x=3, bufs_sm=3, bufs_ysb=3, bufs_s=2,
                 bufs_zb=1, bufs_y=2, skip=(), staggered=True, kt_outer=False,
                 act_copies=4, exp_split=False, load_split=False):
    nc = bacc.Bacc("TRN2", target_bir_lowering=False, debug=False,
                   num_devices=N_CORES)

    xt_d = nc.dram_tensor("xt", [B_PER_CORE, CHUNKS_PER_B, 128, 4, CHUNK],
                          BF16, kind="ExternalInput").ap()
    wsT_d = nc.dram_tensor("wsT", [B_PER_CORE, 128, 4, 128], BF16,
                           kind="ExternalInput").ap()
    wv2p_d = nc.dram_tensor("wv2p", [B_PER_CORE, 128, D], BF16,
                            kind="ExternalInput").ap()
    maskb_d = nc.dram_tensor("mask_bias", [B_PER_CORE, 128, 1], F32,
                             kind="ExternalInput").ap()
    blk16_d = nc.dram_tensor("blk16", [128, 128], BF16,
                             kind="ExternalInput").ap()
    y_shape = ([B_PER_CORE, CHUNKS_PER_B, 128, 4, CHUNK] if Y_WSTAT
               else [B_PER_CORE, CHUNKS_PER_B, 128, GRPS, D])
    y_d = nc.dram_tensor("y", y_shape, BF16, kind="ExternalOutput").ap()

    from contextlib import ExitStack
    with tile.TileContext(nc) as tc:
        with ExitStack() as stack:
            cp = stack.enter_context(tc.tile_pool(name="const", bufs=1))
            wsT = []
            wv2p = []
            for b in range(B_PER_CORE):
                w = cp.tile([128, 4, 128], BF16, tag=f"wsT{b}")
                nc.scalar.dma_start(w[:], wsT_d[b])
                wsT.append(w)
            for b in range(B_PER_CORE):
                w = cp.tile([128, D], BF16, tag=f"wv2p{b}")
                nc.scalar.dma_start(w[:], wv2p_d[b])
                wv2p.append(w)
            blk16 = cp.tile([128, 128], BF16, tag="blk16")
            nc.scalar.dma_start(blk16[:], blk16_d[:])
            maskb = []
            for b in range(B_PER_CORE):
                m = cp.tile([128, 1], F32, tag=f"maskb{b}")
                nc.scalar.dma_start(m[:], maskb_d[b])
                maskb.append(m)

            # ---------------- main loop ----------------
            with (
                tc.tile_pool(name="m_x", bufs=bufs_x) as mp_x,
                tc.tile_pool(name="m_sm", bufs=bufs_sm) as mp_s,
                tc.tile_pool(name="m_ys", bufs=bufs_ysb) as mp_y,
                tc.tile_pool(name="ps_s", bufs=bufs_s, space="PSUM") as ps_s,
                tc.tile_pool(name="ps_zb", bufs=bufs_zb, space="PSUM") as ps_zb,
                tc.tile_pool(name="ps_y", bufs=bufs_y, space="PSUM") as ps_y,
            ):
                from contextlib import nullcontext
                rep_ctx = (tc.For_i(0, repeat, 1, staggered_reset=staggered)
                           if repeat > 1 else nullcontext())
                with rep_ctx:
                    for b in range(B_PER_CORE):
                        for ci in range(CHUNKS_PER_B):
                            xT = mp_x.tile([128, 4, CHUNK], BF16, tag="xT")
                            if "load" not in skip:
                                if load_split:
                                    # two 0.5MB halves: kt 0-1 S-matmuls can
                                    # start while kt 2-3 still lands
                                    nc.sync.dma_start(xT[:, 0:2, :],
                                                      xt_d[b, ci, :, 0:2, :])
                                    nc.sync.dma_start(xT[:, 2:4, :],
                                                      xt_d[b, ci, :, 2:4, :])
                                else:
                                    nc.sync.dma_start(xT[:], xt_d[b, ci])
                            else:
                                nc.vector.memset(xT[:, 0, 0:4], 0.0)

                            # S^T = W_s @ x^T  [128 ch, 1024 n] f32 PSUM
                            s_ps = ps_s.tile([128, CHUNK], F32, tag="s_ps")
                            if kt_outer:
                                # each wsT k-tile stationary for 2 matmuls
                                for kt in range(4):
                                    for hf in range(2):
                                        sl = slice(hf * 512, (hf + 1) * 512)
                                        nc.tensor.matmul(
                                            s_ps[:, sl], wsT[b][:, kt, :],
                                            xT[:, kt, sl],
                                            start=(kt == 0), stop=(kt == 3))
                            else:
                                for hf in range(2):
                                    sl = slice(hf * 512, (hf + 1) * 512)
                                    for kt in range(4):
                                        nc.tensor.matmul(
                                            s_ps[:, sl], wsT[b][:, kt, :],
                                            xT[:, kt, sl],
                                            start=(kt == 0), stop=(kt == 3))

                            # E = exp(S + mask_bias), bf16
                            e_r = mp_s.tile([128, CHUNK], BF16, tag="e_r")
                            if exp_split:
                                # per-half exp: half 0 can run as soon as its
                                # 4 S-matmuls finish, so the zb chain starts
                                # ~4 matmuls earlier
                                for hf in range(2):
                                    sl = slice(hf * 512, (hf + 1) * 512)
                                    nc.scalar.activation(
                                        e_r[:, sl], s_ps[:, sl],
                                        mybir.ActivationFunctionType.Exp,
                                        bias=maskb[b][:], scale=1.0)
                            else:
                                nc.scalar.activation(
                                    e_r[:], s_ps[:],
                                    mybir.ActivationFunctionType.Exp,
                                    bias=maskb[b][:], scale=1.0)

                            a_r = mp_s.tile([128, CHUNK], BF16, tag="a_r")
                            if "softmax" not in skip:
                                # Zrep[ch, n] = per-head sum of E, replicated
                                zb_ps = ps_zb.tile([128, CHUNK], F32,
                                                   tag="zb_ps")
                                for hf in range(2):
                                    sl = slice(hf * 512, (hf + 1) * 512)
                                    nc.tensor.matmul(zb_ps[:, sl], blk16[:],
                                                     e_r[:, sl],
                                                     start=True, stop=True)
                                rzb = mp_s.tile([128, CHUNK], F32, tag="rzb")
                                nc.vector.reciprocal_approx_fast(rzb[:],
                                                                 zb_ps[:])
                                # A = E * recip(Zrep)
                                nc.vector.tensor_mul(a_r[:], e_r[:], rzb[:])
                            else:
                                nc.vector.tensor_copy(a_r[:], e_r[:])

                            act_set = {round(i * GRPS / act_copies)
                                       for i in range(act_copies)} \
                                if act_copies > 0 else set()
                            if Y_WSTAT:
                                # y^T[dout, n] = W_v2p^T @ A: wv2p dt-slice is
                                # stationary for both chunk halves (half the
                                # LDWEIGHTS traffic), a_r is the moving operand
                                y_sb = mp_y.tile([128, 4, CHUNK], BF16,
                                                 tag="y_sb")
                                for dt in range(4):
                                    wsl = wv2p[b][:, dt * 128:(dt + 1) * 128]
                                    for hf in range(2):
                                        g = dt * 2 + hf
                                        sl = slice(hf * 512, (hf + 1) * 512)
                                        y_ps = ps_y.tile([128, D], F32,
                                                         tag="y_ps")
                                        nc.tensor.matmul(y_ps[:], wsl,
                                                         a_r[:, sl],
                                                         start=True, stop=True)
                                        if "ycopy" in skip and g >= 2:
                                            continue
                                        if g in act_set:
                                            nc.scalar.copy(
                                                y_sb[:, dt, sl], y_ps[:])
                                        else:
                                            nc.vector.tensor_copy(
                                                y_sb[:, dt, sl], y_ps[:])
                            else:
                                # y[n-grp g] = A[:, g].T @ W_v2p -> [128 n, 512]
                                y_sb = mp_y.tile([128, GRPS, D], BF16,
                                                 tag="y_sb")
                                for g in range(GRPS):
                                    y_ps = ps_y.tile([128, D], F32, tag="y_ps")
                                    nc.tensor.matmul(
                                        y_ps[:], a_r[:, g * 128:(g + 1) * 128],
                                        wv2p[b][:], start=True, stop=True)
                                    # `act_copies` of 8 go to ScalarE (from
                                    # g=0), spread evenly; rest to VectorE
                                    if "ycopy" in skip and g >= 2:
                                        continue
                                    if g in act_set:
                                        nc.scalar.copy(y_sb[:, g, :], y_ps[:])
                                    else:
                                        nc.vector.tensor_copy(y_sb[:, g, :],
                                                              y_ps[:])

                            if "store" not in skip:
                                nc.scalar.dma_start(y_d[b, ci], y_sb[:])

    nc.compile()
    return nc


def _build_skew(repeat=1, bufs_x=3, bufs_sm=4, bufs_ysb=3, bufs_s=2,
                bufs_zb=1, bufs_y=2, skip=(), staggered=True, act_copies=4):
    """Two-deep software-pipelined variant: emission iteration i runs
    S-matmuls(i) / exp(i), softmax zb+recip+mul(i-1), Y-matmuls+copies+
    store(i-2). Every engine-stream instruction then depends only on work
    from previous emission iterations, so the in-order PE/ACT/DVE queues
    never stall on the intra-chunk exp->zb->recip->mul->Y chain."""
    nc = bacc.Bacc("TRN2", target_bir_lowering=False, debug=False,
                   num_devices=N_CORES)

    xt_d = nc.dram_tensor("xt", [B_PER_CORE, CHUNKS_PER_B, 128, 4, CHUNK],
                          BF16, kind="ExternalInput").ap()
    wsT_d = nc.dram_tensor("wsT", [B_PER_CORE, 128, 4, 128], BF16,
                           kind="ExternalInput").ap()
    wv2p_d = nc.dram_tensor("wv2p", [B_PER_CORE, 128, D], BF16,
                            kind="ExternalInput").ap()
    maskb_d = nc.dram_tensor("mask_bias", [B_PER_CORE, 128, 1], F32,
                             kind="ExternalInput").ap()
    blk16_d = nc.dram_tensor("blk16", [128, 128], BF16,
                             kind="ExternalInput").ap()
    assert not Y_WSTAT, "skew builder only implements the n-major y layout"
    y_d = nc.dram_tensor("y", [B_PER_CORE, CHUNKS_PER_B, 128, GRPS, D],
                         BF16, kind="ExternalOutput").ap()

    from contextlib import ExitStack
    with tile.TileContext(nc) as tc:
        with ExitStack() as stack:
            cp = stack.enter_context(tc.tile_pool(name="const", bufs=1))
            wsT = []
            wv2p = []
            for b in range(B_PER_CORE):
                w = cp.tile([128, 4, 128], BF16, tag=f"wsT{b}")
                nc.scalar.dma_start(w[:], wsT_d[b])
                wsT.append(w)
            for b in range(B_PER_CORE):
                w = cp.tile([128, D], BF16, tag=f"wv2p{b}")
                nc.scalar.dma_start(w[:], wv2p_d[b])
                wv2p.append(w)
            blk16 = cp.tile([128, 128], BF16, tag="blk16")
            nc.scalar.dma_start(blk16[:], blk16_d[:])
            maskb = []
            for b in range(B_PER_CORE):
                m = cp.tile([128, 1], F32, tag=f"maskb{b}")
                nc.scalar.dma_start(m[:], maskb_d[b])
                maskb.append(m)

            chunks = [(b, ci) for b in range(B_PER_CORE)
                      for ci in range(CHUNKS_PER_B)]
            n_ch = len(chunks)
            act_set = {round(i * GRPS / act_copies)
                       for i in range(act_copies)} if act_copies > 0 else set()

            with (
                tc.tile_pool(name="m_x", bufs=bufs_x) as mp_x,
                tc.tile_pool(name="m_sm", bufs=bufs_sm) as mp_s,
                tc.tile_pool(name="m_ys", bufs=bufs_ysb) as mp_y,
                tc.tile_pool(name="ps_s", bufs=bufs_s, space="PSUM") as ps_s,
                tc.tile_pool(name="ps_zb", bufs=bufs_zb, space="PSUM") as ps_zb,
                tc.tile_pool(name="ps_y", bufs=bufs_y, space="PSUM") as ps_y,
            ):
                from contextlib import nullcontext
                rep_ctx = (tc.For_i(0, repeat, 1, staggered_reset=staggered)
                           if repeat > 1 else nullcontext())
                with rep_ctx:
                    state = {}
                    for idx in range(n_ch + 2):
                        # ---- stage A: load + S + exp for chunk idx ----
                        if idx < n_ch:
                            b, ci = chunks[idx]
                            xT = mp_x.tile([128, 4, CHUNK], BF16, tag="xT")
                            if "load" not in skip:
                                nc.sync.dma_start(xT[:], xt_d[b, ci])
                            else:
                                nc.vector.memset(xT[:, 0, 0:4], 0.0)
                            s_ps = ps_s.tile([128, CHUNK], F32, tag="s_ps")
                            for hf in range(2):
                                sl = slice(hf * 512, (hf + 1) * 512)
                                for kt in range(4):
                                    nc.tensor.matmul(
                                        s_ps[:, sl], wsT[b][:, kt, :],
                                        xT[:, kt, sl],
                                        start=(kt == 0), stop=(kt == 3))
                            e_r = mp_s.tile([128, CHUNK], BF16, tag="e_r")
                            nc.scalar.activation(
                                e_r[:], s_ps[:],
                                mybir.ActivationFunctionType.Exp,
                                bias=maskb[b][:], scale=1.0)
                            state[idx] = dict(b=b, ci=ci, e_r=e_r)

                        # ---- stage B: zb + recip + mul for chunk idx-1 ----
                        if 0 <= idx - 1 < n_ch:
                            st = state[idx - 1]
                            e1 = st["e_r"]
                            zb_ps = ps_zb.tile([128, CHUNK], F32, tag="zb_ps")
                            for hf in range(2):
                                sl = slice(hf * 512, (hf + 1) * 512)
                                nc.tensor.matmul(zb_ps[:, sl], blk16[:],
                                                 e1[:, sl],
                                                 start=True, stop=True)
                            a_r = mp_s.tile([128, CHUNK], BF16, tag="a_r")
                            if "softmax" not in skip:
                                rzb = mp_s.tile([128, CHUNK], F32, tag="rzb")
                                nc.vector.reciprocal_approx_fast(rzb[:],
                                                                 zb_ps[:])
                                nc.vector.tensor_mul(a_r[:], e1[:], rzb[:])
                            else:
                                nc.vector.tensor_copy(a_r[:], e1[:])
                            st["a_r"] = a_r

                        # ---- stage C: Y + copies + store for chunk idx-2 ----
                        if idx - 2 >= 0:
                            st = state.pop(idx - 2)
                            b2 = st["b"]
                            a2 = st["a_r"]
                            y_sb = mp_y.tile([128, GRPS, D], BF16, tag="y_sb")
                            for g in range(GRPS):
                                y_ps = ps_y.tile([128, D], F32, tag="y_ps")
                                nc.tensor.matmul(
                                    y_ps[:], a2[:, g * 128:(g + 1) * 128],
                                    wv2p[b2][:], start=True, stop=True)
                                if "ycopy" in skip and g >= 2:
                                    continue
                                if g in act_set:
                                    nc.scalar.copy(y_sb[:, g, :], y_ps[:])
                                else:
                                    nc.vector.tensor_copy(y_sb[:, g, :],
                                                          y_ps[:])
                            if "store" not in skip:
                                nc.scalar.dma_start(y_d[st["b"], st["ci"]],
                                                    y_sb[:])

    nc.compile()
    return nc


def _prep_inputs(x, conditions, condition_mask, w_q, w_kv, w_proj, b_proj):
    """Host-side marshalling: shard over B, fold per-batch weights (f64),
    cast everything to bf16, pre-transpose x into chunk-major x^T layout."""
    x = np.asarray(x, dtype=np.float32)
    conditions = np.asarray(conditions, dtype=np.float64)
    condition_mask = np.asarray(condition_mask)
    w_q = np.asarray(w_q, dtype=np.float64)
    w_kv = np.asarray(w_kv, dtype=np.float64)
    w_proj = np.asarray(w_proj, dtype=np.float64)
    b_proj = np.asarray(b_proj, dtype=np.float64)

    # kv projection for all batches: [B, C, 2, H, hd]
    kv = (conditions @ w_kv.T).reshape(B, C, 2, H, HD)
    k = kv[:, :, 0]    # [B, C, H, hd]
    v = kv[:, :, 1]

    blk16 = np.zeros((128, 128), dtype=NP_BF16)
    for h in range(H):
        blk16[h * C:(h + 1) * C, h * C:(h + 1) * C] = 1.0

    wsT_all = np.zeros((B, 128, 4, 128), dtype=NP_BF16)
    wv2p_all = np.zeros((B, 128, D), dtype=NP_BF16)
    for b in range(B):
        K_blk = np.zeros((128, D))
        V_blk = np.zeros((128, D))
        for h in range(H):
            K_blk[h * C:(h + 1) * C, h * HD:(h + 1) * HD] = k[b, :, h, :]
            V_blk[h * C:(h + 1) * C, h * HD:(h + 1) * HD] = v[b, :, h, :]
        W_s = SCALE * (K_blk @ w_q)             # [ch, ki]
        # lhsT tile layout [ki_in_tile, kt, ch]: wsT[p, t, c] = W_s[c, t*128+p]
        wsT_all[b] = W_s.T.reshape(4, 128, 128).transpose(1, 0, 2).astype(
            NP_BF16)
        wv2p_all[b] = (V_blk @ w_proj.T + b_proj[None, :] / H).astype(NP_BF16)

    # x^T chunks: xt[b, ci, p, kt, n] = x[b, ci*CHUNK + n, kt*128 + p], bf16
    xt_all = np.ascontiguousarray(
        x.reshape(B, CHUNKS_PER_B, CHUNK, 4, 128).transpose(0, 1, 4, 3, 2)
    ).astype(NP_BF16)

    in_maps = []
    for core in range(N_CORES):
        b0 = core * B_PER_CORE
        mb = np.zeros((B_PER_CORE, 128, 1), dtype=np.float32)
        for b in range(B_PER_CORE):
            m = condition_mask[b0 + b].astype(bool)          # [16]
            col = np.where(np.tile(m, H), 0.0, NEG).astype(np.float32)
            mb[b, :, 0] = col
        in_maps.append(dict(
            xt=np.ascontiguousarray(xt_all[b0:b0 + B_PER_CORE]),
            wsT=np.ascontiguousarray(wsT_all[b0:b0 + B_PER_CORE]),
            wv2p=np.ascontiguousarray(wv2p_all[b0:b0 + B_PER_CORE]),
            mask_bias=mb,
            blk16=blk16,
        ))
    return in_maps


def _gather(results):
    """Assemble per-core device y layouts back into [B, N, D] f32."""
    y = np.concatenate([r["y"] for r in results], axis=0)
    if Y_WSTAT:
        # y_dev[b, ci, p, dt, n] = y[b, ci*CHUNK + n, dt*128 + p]
        y = y.astype(np.float32).transpose(0, 1, 4, 3, 2).reshape(B, N, D)
    else:
        # y_dev[b, ci, p, g, d] = y[b, ci*CHUNK + g*128 + p, d]
        y = y.astype(np.float32).transpose(0, 1, 3, 2, 4).reshape(B, N, D)
    return np.ascontiguousarray(y)


def kernel(x, conditions, condition_mask, w_q, w_kv, w_proj, b_proj):
    repeat = int(os.environ.get("MCCA_REPEAT", "1"))
    key = ("nc", repeat)
    if key not in _cache:
        _cache[key] = _build(repeat=repeat)
    nc = _cache[key]
    in_maps = _prep_inputs(x, conditions, condition_mask, w_q, w_kv,
                           w_proj, b_proj)
    res = run_bass_kernel_spmd(nc, in_maps, core_ids=list(range(N_CORES)))
    return _gather(res.results)


# revision 30
# speedup vs baseline: 1.2823x; 1.0437x over previous
"""MultiConditionCrossAttention Trainium2 kernel (8 NeuronCores, data-parallel over B).

Math (per batch b):
    q = x @ w_q.T                                  (B, N, 512)
    kv = conditions @ w_kv.T -> k, v               (B, C=16, H=8, hd=64)
    S = einsum('nhd,chd->hnc', q, k) * SCALE       masked softmax over c
    out = einsum('hnc,chd->nhd', attn, v) @ w_proj.T + b_proj

Key restructuring (exact algebra, see kernel_baseline_222us.py.bak for the
f32 ancestor):
  - Block layouts: K_blk[16h+c, :] = k[c,h,:] placed in head-h's 64-col slice
    (zeros elsewhere); V_blk likewise. Then for all heads at once:
        S_all[n, 16h+c] = q[n] @ K_blk[16h+c]        (block-diag trick)
        out[n]          = attn_all[n] @ V_blk @ w_proj.T + b
  - Weight folding (HOST, float64; q only feeds S, V_blk only feeds the
    projection):
        W_s   = SCALE * K_blk @ w_q          [128, 512]   (per b)
        W_v2p = V_blk @ w_proj.T + b_proj/8  [128, 512]   (per b)
    using sum_ch attn_all[n, ch] = H = 8 to fold the bias exactly.
  - bf16 I/O (the big change vs the f32 baseline): x is cast to bf16 AND
    pre-transposed on the host into chunk-major x^T layout
    xt[b, ci, p, kt, n] = x[b, ci*1024+n, kt*128+p], so the device does NO
    transposes at all and loads half the bytes. y is stored bf16 and
    widened on the host. All weights are bf16; accumulation stays f32 in
    PSUM. Numpy-simulated end-to-end rel err of the full bf16 pipeline:
    4.5e-3 vs the fp32 jax reference (gate is 2e-2); measured on HW:
    4.52e-3. This halves DMA traffic 64MB -> 32MB per core per iteration;
    the f32 baseline was DMA-roofline-bound at ~223us (~300GB/s effective
    of the ~358GB/s per-core HBM limit).

Per 1024-token chunk (16 chunks per core per iteration):
    load xT bf16 [128, 4, 1024]            (1 MB DMA, sync/HWDGE)
    S^T  = W_s @ x^T   (8 bf16 matmuls into [128,1024] f32 PSUM, K=512)
    E    = exp(S^T + mask_bias)            (1 ACT op, PSUM->SBUF bf16)
    Zrep = blk16.T @ E                     (2 matmuls -> per-head sums)
    A    = E * recip_approx(Zrep)          (DVE recip f32 + mul)
    y    = A^T n-slices @ W_v2p            (8 matmuls + 8 PSUM->SBUF copies)
    store y bf16 [128, 8, 512]             (1 MB DMA, scalar/HWDGE)

Measured (8x trn2 NeuronCores, axon; device-resident-input For_i slope,
R=513->2049, see fastbench.py): 120.2us/iteration steady state with the
default knobs (act_copies=5, bufs_s=1, bufs_y=2, bufs_x=4, bufs_sm=4),
vs 222.9us for the f32 baseline (kernel_baseline_222us.py.bak).

HW attribution (For_i slope of skip-variants, default-era config):
full 131.8 / no-store 115.8 / no-softmax 125.9 / compute-only 109.2 /
compute-only minus 6 of 8 y-copies 80.0 us. I.e. after bf16 halves the
DMA, the PSUM->SBUF y-copy + exp/recip/mul pipeline on ACT+DVE is
co-critical with DMA, and most of the win beyond that came from PSUM
pressure (bufs_s 2->1) + copy rebalance (act_copies 4->5).

Negative results (measured, don't retry blindly): 2-deep software skew
(PE stream S(i), zb(i-1), Y(i-2)) 141.5us vs 120.2 plain — same
conclusion as the f32 session's 1-deep skew; sim (TimelineSim) deltas do
NOT transfer to HW (sim favored bufs_s=1+bufs_y=4 which HW ranks worse
than bufs_s=1+bufs_y=2); Y-matmuls with wv2p stationary (y^T layout,
halves LDWEIGHTS) is a wash (129.0 vs 129.2); deeper SBUF pools beyond
(x4, sm4, ysb3) are a wash (122.1); fp8 x fails the rel-err gate
(2.9e-2 numpy-sim vs 2e-2 gate). Filling PSUM hurts monotonically:
bufs_y=3 128.1, bufs_zb=2 148.5 — keep >=2 spare banks. exp_split
(per-half exp to shorten the S->zb chain) 122.6; load_split+kt_outer
(two 0.5MB x loads so kt0-1 matmuls start early) 120.8 — both within
noise of plain 120.2, the extra op overheads cancel the chain gains.
"""

import os
import numpy as np
import ml_dtypes

import concourse.mybir as mybir
import concourse.tile as tile
from concourse import bacc
from concourse.bass_utils import run_bass_kernel_spmd

F32 = mybir.dt.float32
BF16 = mybir.dt.bfloat16
NP_BF16 = ml_dtypes.bfloat16

N_CORES = 8
B, N, D = 16, 8192, 512
C, H, HD = 16, 8, 64
COND_DIM = 256
SCALE = (D // H) ** -0.5
B_PER_CORE = B // N_CORES          # 2
CHUNK = 1024                       # tokens per chunk
CHUNKS_PER_B = N // CHUNK          # 8
GRPS = CHUNK // 128                # 8 n-groups per chunk
NEG = -60.0                        # mask bias (exp(-60+s) ~ 0)

# Y-matmul orientation: False = a_r slices stationary, y n-major
# (y_dev[b,ci,p,g,d], token = ci*CHUNK + g*128 + p);  True = wv2p slices
# stationary (reused across both chunk halves), y feature-major
# (y_dev[b,ci,p,dt,n], y[b, ci*CHUNK+n, dt*128+p]).
Y_WSTAT = os.environ.get("MCCA_YWSTAT", "0") == "1"

_cache = {}


def _build(repeat=1, bufs_x=4, bufs_sm=4, bufs_ysb=3, bufs_s=1, bufs_zb=1,
           bufs_y=2, skip=(), staggered=True, kt_outer=False, act_copies=5,
           skew=False, exp_split=False, load_split=False, sm_split=False):
    if skew:
        return _build_skew(repeat=repeat, bufs_x=bufs_x, bufs_sm=bufs_sm,
                           bufs_ysb=bufs_ysb, bufs_s=bufs_s, bufs_zb=bufs_zb,
                           bufs_y=bufs_y, skip=skip, staggered=staggered,
                           act_copies=act_copies)
    return _build_plain(repeat=repeat, bufs_x=bufs_x, bufs_sm=bufs_sm,
                        bufs_ysb=bufs_ysb, bufs_s=bufs_s, bufs_zb=bufs_zb,
                        bufs_y=bufs_y, skip=skip, staggered=staggered,
                        kt_outer=kt_outer, act_copies=act_copies,
                        exp_split=exp_split, load_split=load_split,
                        sm_split=sm_split)


def _build_plain(repeat=1, bufs_x=3, bufs_sm=3, bufs_ysb=3, bufs_s=2,
                 bufs_zb=1, bufs_y=2, skip=(), staggered=True, kt_outer=False,
                 act_copies=4, exp_split=False, load_split=False,
                 sm_split=False):
    nc = bacc.Bacc("TRN2", target_bir_lowering=False, debug=False,
                   num_devices=N_CORES)

    xt_d = nc.dram_tensor("xt", [B_PER_CORE, CHUNKS_PER_B, 128, 4, CHUNK],
                          BF16, kind="ExternalInput").ap()
    wsT_d = nc.dram_tensor("wsT", [B_PER_CORE, 128, 4, 128], BF16,
                           kind="ExternalInput").ap()
    wv2p_d = nc.dram_tensor("wv2p", [B_PER_CORE, 128, D], BF16,
                            kind="ExternalInput").ap()
    maskb_d = nc.dram_tensor("mask_bias", [B_PER_CORE, 128, 1], F32,
                             kind="ExternalInput").ap()
    blk16_d = nc.dram_tensor("blk16", [128, 128], BF16,
                             kind="ExternalInput").ap()
    y_shape = ([B_PER_CORE, CHUNKS_PER_B, 128, 4, CHUNK] if Y_WSTAT
               else [B_PER_CORE, CHUNKS_PER_B, 128, GRPS, D])
    y_d = nc.dram_tensor("y", y_shape, BF16, kind="ExternalOutput").ap()

    from contextlib import ExitStack
    with tile.TileContext(nc) as tc:
        with ExitStack() as stack:
            cp = stack.enter_context(tc.tile_pool(name="const", bufs=1))
            wsT = []
            wv2p = []
            for b in range(B_PER_CORE):
                w = cp.tile([128, 4, 128], BF16, tag=f"wsT{b}")
                nc.scalar.dma_start(w[:], wsT_d[b])
                wsT.append(w)
            for b in range(B_PER_CORE):
                w = cp.tile([128, D], BF16, tag=f"wv2p{b}")
                nc.scalar.dma_start(w[:], wv2p_d[b])
                wv2p.append(w)
            blk16 = cp.tile([128, 128], BF16, tag="blk16")
            nc.scalar.dma_start(blk16[:], blk16_d[:])
            maskb = []
            for b in range(B_PER_CORE):
                m = cp.tile([128, 1], F32, tag=f"maskb{b}")
                nc.scalar.dma_start(m[:], maskb_d[b])
                maskb.append(m)

            # ---------------- main loop ----------------
            with (
                tc.tile_pool(name="m_x", bufs=bufs_x) as mp_x,
                tc.tile_pool(name="m_sm", bufs=bufs_sm) as mp_s,
                tc.tile_pool(name="m_ys", bufs=bufs_ysb) as mp_y,
                tc.tile_pool(name="ps_s", bufs=bufs_s, space="PSUM") as ps_s,
                tc.tile_pool(name="ps_zb", bufs=bufs_zb, space="PSUM") as ps_zb,
                tc.tile_pool(name="ps_y", bufs=bufs_y, space="PSUM") as ps_y,
            ):
                from contextlib import nullcontext
                rep_ctx = (tc.For_i(0, repeat, 1, staggered_reset=staggered)
                           if repeat > 1 else nullcontext())
                with rep_ctx:
                    for b in range(B_PER_CORE):
                        for ci in range(CHUNKS_PER_B):
                            xT = mp_x.tile([128, 4, CHUNK], BF16, tag="xT")
                            if "load" not in skip:
                                if load_split:
                                    # two 0.5MB halves: kt 0-1 S-matmuls can
                                    # start while kt 2-3 still lands
                                    nc.sync.dma_start(xT[:, 0:2, :],
                                                      xt_d[b, ci, :, 0:2, :])
                                    nc.sync.dma_start(xT[:, 2:4, :],
                                                      xt_d[b, ci, :, 2:4, :])
                                else:
                                    nc.sync.dma_start(xT[:], xt_d[b, ci])
                            else:
                                nc.vector.memset(xT[:, 0, 0:4], 0.0)

                            # S^T = W_s @ x^T  [128 ch, 1024 n] f32 PSUM
                            s_ps = ps_s.tile([128, CHUNK], F32, tag="s_ps")
                            if kt_outer:
                                # each wsT k-tile stationary for 2 matmuls
                                for kt in range(4):
                                    for hf in range(2):
                                        sl = slice(hf * 512, (hf + 1) * 512)
                                        nc.tensor.matmul(
                                            s_ps[:, sl], wsT[b][:, kt, :],
                                            xT[:, kt, sl],
                                            start=(kt == 0), stop=(kt == 3))
                            else:
                                for hf in range(2):
                                    sl = slice(hf * 512, (hf + 1) * 512)
                                    for kt in range(4):
                                        nc.tensor.matmul(
                                            s_ps[:, sl], wsT[b][:, kt, :],
                                            xT[:, kt, sl],
                                            start=(kt == 0), stop=(kt == 3))

                            # E = exp(S + mask_bias), bf16
                            e_r = mp_s.tile([128, CHUNK], BF16, tag="e_r")
                            if exp_split:
                                # per-half exp: half 0 can run as soon as its
                                # 4 S-matmuls finish, so the zb chain starts
                                # ~4 matmuls earlier
                                for hf in range(2):
                                    sl = slice(hf * 512, (hf + 1) * 512)
                                    nc.scalar.activation(
                                        e_r[:, sl], s_ps[:, sl],
                                        mybir.ActivationFunctionType.Exp,
                                        bias=maskb[b][:], scale=1.0)
                            else:
                                nc.scalar.activation(
                                    e_r[:], s_ps[:],
                                    mybir.ActivationFunctionType.Exp,
                                    bias=maskb[b][:], scale=1.0)

                            a_r = mp_s.tile([128, CHUNK], BF16, tag="a_r")
                            if "softmax" not in skip:
                                # Zrep[ch, n] = per-head sum of E, replicated
                                zb_ps = ps_zb.tile([128, CHUNK], F32,
                                                   tag="zb_ps")
                                for hf in range(2):
                                    sl = slice(hf * 512, (hf + 1) * 512)
                                    nc.tensor.matmul(zb_ps[:, sl], blk16[:],
                                                     e_r[:, sl],
                                                     start=True, stop=True)
                                rzb = mp_s.tile([128, CHUNK], F32, tag="rzb")
                                if sm_split:
                                    # per-half recip+mul: Y matmuls g0-3 can
                                    # start after half the DVE chain
                                    for hf in range(2):
                                        sl = slice(hf * 512, (hf + 1) * 512)
                                        nc.vector.reciprocal_approx_fast(
                                            rzb[:, sl], zb_ps[:, sl])
                                        nc.vector.tensor_mul(
                                            a_r[:, sl], e_r[:, sl],
                                            rzb[:, sl])
                                else:
                                    nc.vector.reciprocal_approx_fast(
                                        rzb[:], zb_ps[:])
                                    # A = E * recip(Zrep)
                                    nc.vector.tensor_mul(a_r[:], e_r[:],
                                                         rzb[:])
                            else:
                                nc.vector.tensor_copy(a_r[:], e_r[:])

                            act_set = {round(i * GRPS / act_copies)
                                       for i in range(act_copies)} \
                                if act_copies > 0 else set()
                            if Y_WSTAT:
                                # y^T[dout, n] = W_v2p^T @ A: wv2p dt-slice is
                                # stationary for both chunk halves (half the
                                # LDWEIGHTS traffic), a_r is the moving operand
                                y_sb = mp_y.tile([128, 4, CHUNK], BF16,
                                                 tag="y_sb")
                                for dt in range(4):
                                    wsl = wv2p[b][:, dt * 128:(dt + 1) * 128]
                                    for hf in range(2):
                                        g = dt * 2 + hf
                                        sl = slice(hf * 512, (hf + 1) * 512)
                                        y_ps = ps_y.tile([128, D], F32,
                                                         tag="y_ps")
                                        nc.tensor.matmul(y_ps[:], wsl,
                                                         a_r[:, sl],
                                                         start=True, stop=True)
                                        if "ycopy" in skip and g >= 2:
                                            continue
                                        if g in act_set:
                                            nc.scalar.copy(
                                                y_sb[:, dt, sl], y_ps[:])
                                        else:
                                            nc.vector.tensor_copy(
                                                y_sb[:, dt, sl], y_ps[:])
                            else:
                                # y[n-grp g] = A[:, g].T @ W_v2p -> [128 n, 512]
                                y_sb = mp_y.tile([128, GRPS, D], BF16,
                                                 tag="y_sb")
                                for g in range(GRPS):
                                    y_ps = ps_y.tile([128, D], F32, tag="y_ps")
                                    nc.tensor.matmul(
                                        y_ps[:], a_r[:, g * 128:(g + 1) * 128],
                                        wv2p[b][:], start=True, stop=True)
                                    # `act_copies` of 8 go to ScalarE (from
                                    # g=0), spread evenly; rest to VectorE
                                    if "ycopy" in skip and g >= 2:
                                        continue
                                    if g in act_set:
                                        nc.scalar.copy(y_sb[:, g, :], y_ps[:])
                                    else:
                                        nc.vector.tensor_copy(y_sb[:, g, :],
                                                              y_ps[:])

                            if "store" not in skip:
                                nc.scalar.dma_start(y_d[b, ci], y_sb[:])

    nc.compile()
    return nc


def _build_skew(repeat=1, bufs_x=3, bufs_sm=4, bufs_ysb=3, bufs_s=2,
                bufs_zb=1, bufs_y=2, skip=(), staggered=True, act_copies=4):
    """Two-deep software-pipelined variant: emission iteration i runs
    S-matmuls(i) / exp(i), softmax zb+recip+mul(i-1), Y-matmuls+copies+
    store(i-2). Every engine-stream instruction then depends only on work
    from previous emission iterations, so the in-order PE/ACT/DVE queues
    never stall on the intra-chunk exp->zb->recip->mul->Y chain."""
    nc = bacc.Bacc("TRN2", target_bir_lowering=False, debug=False,
                   num_devices=N_CORES)

    xt_d = nc.dram_tensor("xt", [B_PER_CORE, CHUNKS_PER_B, 128, 4, CHUNK],
                          BF16, kind="ExternalInput").ap()
    wsT_d = nc.dram_tensor("wsT", [B_PER_CORE, 128, 4, 128], BF16,
                           kind="ExternalInput").ap()
    wv2p_d = nc.dram_tensor("wv2p", [B_PER_CORE, 128, D], BF16,
                            kind="ExternalInput").ap()
    maskb_d = nc.dram_tensor("mask_bias", [B_PER_CORE, 128, 1], F32,
                             kind="ExternalInput").ap()
    blk16_d = nc.dram_tensor("blk16", [128, 128], BF16,
                             kind="ExternalInput").ap()
    assert not Y_WSTAT, "skew builder only implements the n-major y layout"
    y_d = nc.dram_tensor("y", [B_PER_CORE, CHUNKS_PER_B, 128, GRPS, D],
                         BF16, kind="ExternalOutput").ap()

    from contextlib import ExitStack
    with tile.TileContext(nc) as tc:
        with ExitStack() as stack:
            cp = stack.enter_context(tc.tile_pool(name="const", bufs=1))
            wsT = []
            wv2p = []
            for b in range(B_PER_CORE):
                w = cp.tile([128, 4, 128], BF16, tag=f"wsT{b}")
                nc.scalar.dma_start(w[:], wsT_d[b])
                wsT.append(w)
            for b in range(B_PER_CORE):
                w = cp.tile([128, D], BF16, tag=f"wv2p{b}")
                nc.scalar.dma_start(w[:], wv2p_d[b])
                wv2p.append(w)
            blk16 = cp.tile([128, 128], BF16, tag="blk16")
            nc.scalar.dma_start(blk16[:], blk16_d[:])
            maskb = []
            for b in range(B_PER_CORE):
                m = cp.tile([128, 1], F32, tag=f"maskb{b}")
                nc.scalar.dma_start(m[:], maskb_d[b])
                maskb.append(m)

            chunks = [(b, ci) for b in range(B_PER_CORE)
                      for ci in range(CHUNKS_PER_B)]
            n_ch = len(chunks)
            act_set = {round(i * GRPS / act_copies)
                       for i in range(act_copies)} if act_copies > 0 else set()

            with (
                tc.tile_pool(name="m_x", bufs=bufs_x) as mp_x,
                tc.tile_pool(name="m_sm", bufs=bufs_sm) as mp_s,
                tc.tile_pool(name="m_ys", bufs=bufs_ysb) as mp_y,
                tc.tile_pool(name="ps_s", bufs=bufs_s, space="PSUM") as ps_s,
                tc.tile_pool(name="ps_zb", bufs=bufs_zb, space="PSUM") as ps_zb,
                tc.tile_pool(name="ps_y", bufs=bufs_y, space="PSUM") as ps_y,
            ):
                from contextlib import nullcontext
                rep_ctx = (tc.For_i(0, repeat, 1, staggered_reset=staggered)
                           if repeat > 1 else nullcontext())
                with rep_ctx:
                    state = {}
                    for idx in range(n_ch + 2):
                        # ---- stage A: load + S + exp for chunk idx ----
                        if idx < n_ch:
                            b, ci = chunks[idx]
                            xT = mp_x.tile([128, 4, CHUNK], BF16, tag="xT")
                            if "load" not in skip:
                                nc.sync.dma_start(xT[:], xt_d[b, ci])
                            else:
                                nc.vector.memset(xT[:, 0, 0:4], 0.0)
                            s_ps = ps_s.tile([128, CHUNK], F32, tag="s_ps")
                            for hf in range(2):
                                sl = slice(hf * 512, (hf + 1) * 512)
                                for kt in range(4):
                                    nc.tensor.matmul(
                                        s_ps[:, sl], wsT[b][:, kt, :],
                                        xT[:, kt, sl],
                                        start=(kt == 0), stop=(kt == 3))
                            e_r = mp_s.tile([128, CHUNK], BF16, tag="e_r")
                            nc.scalar.activation(
                                e_r[:], s_ps[:],
                                mybir.ActivationFunctionType.Exp,
                                bias=maskb[b][:], scale=1.0)
                            state[idx] = dict(b=b, ci=ci, e_r=e_r)

                        # ---- stage B: zb + recip + mul for chunk idx-1 ----
                        if 0 <= idx - 1 < n_ch:
                            st = state[idx - 1]
                            e1 = st["e_r"]
                            zb_ps = ps_zb.tile([128, CHUNK], F32, tag="zb_ps")
                            for hf in range(2):
                                sl = slice(hf * 512, (hf + 1) * 512)
                                nc.tensor.matmul(zb_ps[:, sl], blk16[:],
                                                 e1[:, sl],
                                                 start=True, stop=True)
                            a_r = mp_s.tile([128, CHUNK], BF16, tag="a_r")
                            if "softmax" not in skip:
                                rzb = mp_s.tile([128, CHUNK], F32, tag="rzb")
                                nc.vector.reciprocal_approx_fast(rzb[:],
                                                                 zb_ps[:])
                                nc.vector.tensor_mul(a_r[:], e1[:], rzb[:])
                            else:
                                nc.vector.tensor_copy(a_r[:], e1[:])
                            st["a_r"] = a_r

                        # ---- stage C: Y + copies + store for chunk idx-2 ----
                        if idx - 2 >= 0:
                            st = state.pop(idx - 2)
                            b2 = st["b"]
                            a2 = st["a_r"]
                            y_sb = mp_y.tile([128, GRPS, D], BF16, tag="y_sb")
                            for g in range(GRPS):
                                y_ps = ps_y.tile([128, D], F32, tag="y_ps")
                                nc.tensor.matmul(
                                    y_ps[:], a2[:, g * 128:(g + 1) * 128],
                                    wv2p[b2][:], start=True, stop=True)
                                if "ycopy" in skip and g >= 2:
                                    continue
                                if g in act_set:
                                    nc.scalar.copy(y_sb[:, g, :], y_ps[:])
                                else:
                                    nc.vector.tensor_copy(y_sb[:, g, :],
                                                          y_ps[:])
                            if "store" not in skip:
                                nc.scalar.dma_start(y_d[st["b"], st["ci"]],
                                                    y_sb[:])

    nc.compile()
    return nc


def _prep_inputs(x, conditions, condition_mask, w_q, w_kv, w_proj, b_proj):
    """Host-side marshalling: shard over B, fold per-batch weights (f64),
    cast everything to bf16, pre-transpose x into chunk-major x^T layout."""
    x = np.asarray(x, dtype=np.float32)
    conditions = np.asarray(conditions, dtype=np.float64)
    condition_mask = np.asarray(condition_mask)
    w_q = np.asarray(w_q, dtype=np.float64)
    w_kv = np.asarray(w_kv, dtype=np.float64)
    w_proj = np.asarray(w_proj, dtype=np.float64)
    b_proj = np.asarray(b_proj, dtype=np.float64)

    # kv projection for all batches: [B, C, 2, H, hd]
    kv = (conditions @ w_kv.T).reshape(B, C, 2, H, HD)
    k = kv[:, :, 0]    # [B, C, H, hd]
    v = kv[:, :, 1]

    blk16 = np.zeros((128, 128), dtype=NP_BF16)
    for h in range(H):
        blk16[h * C:(h + 1) * C, h * C:(h + 1) * C] = 1.0

    wsT_all = np.zeros((B, 128, 4, 128), dtype=NP_BF16)
    wv2p_all = np.zeros((B, 128, D), dtype=NP_BF16)
    for b in range(B):
        K_blk = np.zeros((128, D))
        V_blk = np.zeros((128, D))
        for h in range(H):
            K_blk[h * C:(h + 1) * C, h * HD:(h + 1) * HD] = k[b, :, h, :]
            V_blk[h * C:(h + 1) * C, h * HD:(h + 1) * HD] = v[b, :, h, :]
        W_s = SCALE * (K_blk @ w_q)             # [ch, ki]
        # lhsT tile layout [ki_in_tile, kt, ch]: wsT[p, t, c] = W_s[c, t*128+p]
        wsT_all[b] = W_s.T.reshape(4, 128, 128).transpose(1, 0, 2).astype(
            NP_BF16)
        wv2p_all[b] = (V_blk @ w_proj.T + b_proj[None, :] / H).astype(NP_BF16)

    # x^T chunks: xt[b, ci, p, kt, n] = x[b, ci*CHUNK + n, kt*128 + p], bf16
    xt_all = np.ascontiguousarray(
        x.reshape(B, CHUNKS_PER_B, CHUNK, 4, 128).transpose(0, 1, 4, 3, 2)
    ).astype(NP_BF16)

    in_maps = []
    for core in range(N_CORES):
        b0 = core * B_PER_CORE
        mb = np.zeros((B_PER_CORE, 128, 1), dtype=np.float32)
        for b in range(B_PER_CORE):
            m = condition_mask[b0 + b].astype(bool)          # [16]
            col = np.where(np.tile(m, H), 0.0, NEG).astype(np.float32)
            mb[b, :, 0] = col
        in_maps.append(dict(
            xt=np.ascontiguousarray(xt_all[b0:b0 + B_PER_CORE]),
            wsT=np.ascontiguousarray(wsT_all[b0:b0 + B_PER_CORE]),
            wv2p=np.ascontiguousarray(wv2p_all[b0:b0 + B_PER_CORE]),
            mask_bias=mb,
            blk16=blk16,
        ))
    return in_maps


def _gather(results):
    """Assemble per-core device y layouts back into [B, N, D] f32."""
    y = np.concatenate([r["y"] for r in results], axis=0)
    if Y_WSTAT:
        # y_dev[b, ci, p, dt, n] = y[b, ci*CHUNK + n, dt*128 + p]
        y = y.astype(np.float32).transpose(0, 1, 4, 3, 2).reshape(B, N, D)
    else:
        # y_dev[b, ci, p, g, d] = y[b, ci*CHUNK + g*128 + p, d]
        y = y.astype(np.float32).transpose(0, 1, 3, 2, 4).reshape(B, N, D)
    return np.ascontiguousarray(y)


def kernel(x, conditions, condition_mask, w_q, w_kv, w_proj, b_proj):
    repeat = int(os.environ.get("MCCA_REPEAT", "1"))
    key = ("nc", repeat)
    if key not in _cache:
        _cache[key] = _build(repeat=repeat)
    nc = _cache[key]
    in_maps = _prep_inputs(x, conditions, condition_mask, w_q, w_kv,
                           w_proj, b_proj)
    res = run_bass_kernel_spmd(nc, in_maps, core_ids=list(range(N_CORES)))
    return _gather(res.results)
